# revision 17
# baseline (speedup 1.0000x reference)
# nn_GPT_64347200029289 — RWKV6-style dense transformer on 8 TRN2 NeuronCores.
# B=4, T=1024, C=768, H=12 heads (headdim 64), L=12 layers, V=50304.
# Output: last-position logits [B, 1, V].
#
# Sharding: the host→device tunnel is the bottleneck (~75 MB/s effective), so
# all large weight tensors are shipped 8-way sharded (each core receives a
# distinct 1/8 slice) and reassembled on-device with AllGather collectives
# over NeuronLink. After the gather every core holds the full weights in
# internal DRAM; core c then runs the full 12-layer body for batch c%4 and
# computes full-vocab logits (cores 4-7 duplicate 0-3; host keeps cores 0-3).
#
# Layout: residual kept C-major ([C-tile=128 partitions, T free], fp32).
# All matmuls bf16 with fp32 PSUM accumulation. LayerNorm/GroupNorm affine
# params are folded into adjacent projection weights on the host. LN stats are
# partition reductions via ones-matmuls; GroupNorm stats use block-diagonal
# ones-matmuls so y stays C-major (no transposes anywhere). Attention is the
# chunked RWKV scan (Q=256, 4 chunks) with host-precomputed decay tables.

import sys
import numpy as np

sys.path.insert(0, "/opt/trn_rl_repo")

import ml_dtypes

# Persistent jax compilation cache: run_bass_kernel_spmd builds a fresh
# jax.jit per call, which otherwise re-runs the ~5s neuronxcc compile of the
# (unchanged) NEFF on every call.
try:
    import jax
    jax.config.update("jax_compilation_cache_dir", "/tmp/jaxcache")
    jax.config.update("jax_persistent_cache_min_compile_time_secs", 0.0)
    jax.config.update("jax_persistent_cache_min_entry_size_bytes", 0)
except Exception:
    pass

C, H, L, V, BLK = 768, 12, 12, 50304, 1024
HD = C // H                  # 64
B, T, Q = 4, 1024, 256
NCH = T // Q                 # 4 chunks
CT = C // 128                # 6
TT = T // 128                # 8
HT = (3 * C) // 128          # 18
GN_EPS = 1e-5 * 64
LN_EPS = 1e-5
NCORES = 8

BF = np.float16

# name -> (natural shape, 2D collective shape [rows, cols]); rows % 8 == 0,
# rank-r shard is rows/8 consecutive rows. Gathered tensor layout == natural.
SHARD_SPECS = {
    "Wr":   ((L, 128, CT, C), (L * 128 * CT, C)),
    "Wk":   ((L, 128, CT, C), (L * 128 * CT, C)),
    "Wv":   ((L, 128, CT, C), (L * 128 * CT, C)),
    "Wg":   ((L, 128, CT, C), (L * 128 * CT, C)),
    "Wo":   ((L, 128, CT, C), (L * 128 * CT, C)),
    "Wcr":  ((L, 128, CT, C), (L * 128 * CT, C)),
    "Wck":  ((L, 128, CT, 3 * C), (L * 128 * CT, 3 * C)),
    "Wcv":  ((L, 128, HT, C), (L * 128 * HT, C)),
    "wteT": ((128, CT, V), (128 * CT, V)),
}


def _host_precompute(inputs):
    f = lambda k: np.asarray(inputs[k], np.float32)
    idx = np.asarray(inputs["idx"])
    wte, wpe = f("wte"), f("wpe")
    ln1_w, ln1_b = f("ln1_w"), f("ln1_b")
    ln2_w, ln2_b = f("ln2_w"), f("ln2_b")
    gn_w, gn_b = f("gn_w"), f("gn_b")
    lnf_w, lnf_b = f("lnf_w"), f("lnf_b")
    Wr, Wk, Wv, Wg, Wo = f("Wr"), f("Wk"), f("Wv"), f("Wg"), f("Wo")
    Wck, Wcv, Wcr = f("Wck"), f("Wcv"), f("Wcr")
    br, bk, bv, bg, bo = f("br"), f("bk"), f("bv"), f("bg"), f("bo")
    bck, bcv, bcr = f("bck"), f("bcv"), f("bcr")
    maa_tk, maa_tv = f("maa_tk"), f("maa_tv")
    maa_tr, maa_tg = f("maa_tr"), f("maa_tg")
    cmaa_k, cmaa_r = f("cmaa_k"), f("cmaa_r")
    tdecay, tfaaaa = f("tdecay"), f("tfaaaa")

    def fold(W, lw, lb, bproj):
        We = lw[:, :, None] * W
        be = bproj + np.einsum("lc,lco->lo", lb, W)
        return We, be

    Wr_e, br_e = fold(Wr, ln1_w, ln1_b, br)
    Wk_e, bk_e = fold(Wk, ln1_w, ln1_b, bk)
    Wv_e, bv_e = fold(Wv, ln1_w, ln1_b, bv)
    Wg_e, bg_e = fold(Wg, ln1_w, ln1_b, bg)
    Wck_e, bck_e = fold(Wck, ln2_w, ln2_b, bck)
    Wcr_e, bcr_e = fold(Wcr, ln2_w, ln2_b, bcr)
    Wo_e = gn_w[:, :, None] * Wo
    Wcv_e = Wcv
    bo_e, bcv_e = bo, bcv

    w = np.exp(-np.exp(tdecay)).astype(np.float64)       # [L,H]
    ii = np.arange(Q)
    wk_ = (w[:, :, None] ** (Q - 1 - ii)[None, None, :]).astype(np.float32)
    ws_ = (w ** Q).astype(np.float32)

    wteT_e = lnf_w[:, None] * wte.T                      # [C,V]
    lbias = lnf_b @ wte.T                                # [V]
    x0 = wte[idx] + wpe[:T]                              # [B,T,C]

    def cm(M):  # [Cin,F] -> [128, Cin//128, F]
        Cin, F2 = M.shape
        return np.ascontiguousarray(M.reshape(Cin // 128, 128, F2).transpose(1, 0, 2))

    big = {}
    for name, We in (("Wr", Wr_e), ("Wk", Wk_e), ("Wv", Wv_e), ("Wg", Wg_e),
                     ("Wo", Wo_e), ("Wcr", Wcr_e), ("Wck", Wck_e), ("Wcv", Wcv_e)):
        big[name] = np.stack([cm(We[l]) for l in range(L)]).astype(BF)
    big["wteT"] = cm(wteT_e).astype(BF)                  # [128, CT, V]

    # wmT/wbq decay tables are generated on-device from lnw = ln(w) =
    # -exp(tdecay): wmT[l,p,h,jt,i] = exp(min(lnw*(i-j-1),0))*[i>j] + u*[i==j]
    # with j = jt*128+p; wbq[l,p,ct,i] = exp(lnw_head(ct,p) * i).
    lnw = (-np.exp(tdecay)).astype(np.float32)           # [L,H]
    lnw_h = np.ascontiguousarray(
        np.broadcast_to(lnw[:, None, :], (L, 128, H))).astype(np.float32)
    u_h = np.ascontiguousarray(
        np.broadcast_to(tfaaaa[:, None, :], (L, 128, H))).astype(np.float32)
    lnw_ct = np.zeros((L, 128, CT), np.float32)
    for ct in range(CT):
        lnw_ct[:, 0:64, ct] = lnw[:, 2 * ct, None]
        lnw_ct[:, 64:128, ct] = lnw[:, 2 * ct + 1, None]
    ivec = np.arange(Q, dtype=np.float32)
    jvec = np.arange(128, dtype=np.float32)
    emat = np.zeros((128, 2, Q), np.float32)
    mtri = np.zeros((128, 2, Q), np.float32)
    meye = np.zeros((128, 2, Q), np.float32)
    for jt in range(2):
        jj = jt * 128 + jvec[:, None]
        emat[:, jt, :] = ivec[None, :] - jj - 1.0
        mtri[:, jt, :] = (ivec[None, :] > jj).astype(np.float32)
        meye[:, jt, :] = (ivec[None, :] == jj).astype(np.float32)
    irow = np.broadcast_to(ivec[None, :], (128, Q))

    wk_col = wk_.reshape(L, H, 2, 128).transpose(0, 3, 1, 2).reshape(L, 128, H * 2)
    ws_bc = np.zeros((L, 128, H), np.float32)
    ws_bc[:, 0:64, :] = ws_[:, None, :]
    ws_bc[:, 64:128, :] = ws_[:, None, :]

    # mix coefficients [L,128,CT,12]: kinds tk,tv,tr,tg,ck,cr then negated
    maa_all = np.stack([maa_tk, maa_tv, maa_tr,
                        maa_tg, cmaa_k, cmaa_r], axis=-1)   # [L,C,6]
    maa_all = np.concatenate([maa_all, 1.0 - maa_all], axis=-1)
    maa_pack = maa_all.reshape(L, CT, 128, 12).transpose(0, 2, 1, 3)

    # C-major per-partition biases [L,128,CT,8]: br,bkC,bo,bcv,bcr,gnb,bg,pad
    bias_cm = np.stack([br_e, bk_e, bo_e, bcv_e, bcr_e,
                        np.broadcast_to(gn_b, br_e.shape), bg_e,
                        np.zeros_like(br_e)], axis=-1)
    bias_cm = bias_cm.reshape(L, CT, 128, 8).transpose(0, 2, 1, 3)
    bck_t = bck_e.reshape(L, HT, 128).transpose(0, 2, 1)          # [L,128,HT]
    bias_rows = np.stack([bk_e, bv_e], axis=1)                    # [L,2,C]

    blk_a = np.zeros((128, CT, H), np.float32)
    blk_b = np.zeros((12, CT, 128), np.float32)
    for ct in range(CT):
        blk_a[0:64, ct, 2 * ct] = 1.0
        blk_a[64:128, ct, 2 * ct + 1] = 1.0
        blk_b[2 * ct, ct, 0:64] = 1.0
        blk_b[2 * ct + 1, ct, 64:128] = 1.0

    common = {
        "maa": maa_pack.astype(np.float32),
        "bias_cm": bias_cm.astype(np.float32),
        "bck_t": bck_t.astype(np.float32),
        "bias_rows": bias_rows.astype(BF),
        "wk_col": np.ascontiguousarray(wk_col).astype(np.float32),
        "ws_bc": ws_bc.astype(np.float32),
        "blk_a": blk_a.astype(BF),
        "blk_af": blk_a.astype(np.float32),
        "blk_b": blk_b.astype(np.float32),
        "lnw_h": lnw_h,
        "u_h": u_h,
        "lnw_ct": lnw_ct,
        "emat": emat.astype(BF),
        "mtri": mtri.astype(BF),
        "meye": meye.astype(BF),
        "irow": np.ascontiguousarray(irow).astype(BF),
    }

    # 1/8 row-shards of each big tensor (concat over ranks == natural layout)
    for name, (nat, two_d) in SHARD_SPECS.items():
        rows, cols = two_d
        arr = big[name].reshape(rows, cols)
        rs = rows // NCORES
        big[name] = [np.ascontiguousarray(arr[c * rs:(c + 1) * rs])
                     for c in range(NCORES)]

    in_maps = []
    for c in range(NCORES):
        b = c % 4
        m = dict(common)
        for name in SHARD_SPECS:
            m[name + "_sh"] = big[name][c]
        m["x0"] = np.ascontiguousarray(
            x0[b].T.reshape(CT, 128, T).transpose(1, 0, 2)).astype(BF)
        in_maps.append(m)
    return in_maps, lbias


# ---------------------------------------------------------------------------

_PROG_CACHE = {}


def _build_program(n_layers=L):
    import concourse.bass as bass
    import concourse.tile as tile
    from concourse import mybir, bacc
    from contextlib import ExitStack

    f32 = mybir.dt.float32
    bf16 = mybir.dt.float16
    AF = mybir.ActivationFunctionType
    OP = mybir.AluOpType

    nc = bacc.Bacc("TRN2", target_bir_lowering=False, debug=False,
                   num_devices=NCORES)

    dram = {}
    def din(name, shape, dt=bf16):
        dram[name] = nc.dram_tensor(name, list(shape), dt, kind="ExternalInput")

    din("x0", (128, CT, T))
    for name, (nat, (rows, cols)) in SHARD_SPECS.items():
        din(name + "_sh", (rows // NCORES, cols))
    din("maa", (L, 128, CT, 12), f32)
    din("bias_cm", (L, 128, CT, 8), f32)
    din("bck_t", (L, 128, HT), f32)
    din("bias_rows", (L, 2, C))
    din("wk_col", (L, 128, H * 2), f32)
    din("ws_bc", (L, 128, H), f32)
    din("blk_a", (128, CT, H))
    din("blk_af", (128, CT, H), f32)
    din("blk_b", (12, CT, 128), f32)
    din("lnw_h", (L, 128, H), f32)
    din("u_h", (L, 128, H), f32)
    din("lnw_ct", (L, 128, CT), f32)
    din("emat", (128, 2, Q))
    din("mtri", (128, 2, Q))
    din("meye", (128, 2, Q))
    din("irow", (128, Q))
    out_logits = nc.dram_tensor("logits", [1, V], f32, kind="ExternalOutput")

    with tile.TileContext(nc) as tc:
        with ExitStack() as ctx:
            dpool = ctx.enter_context(tc.tile_pool(name="dpool", bufs=1,
                                                   space="DRAM"))
            pers = ctx.enter_context(tc.tile_pool(name="pers", bufs=1))
            lcon = ctx.enter_context(tc.tile_pool(name="lcon", bufs=1))
            wp = ctx.enter_context(tc.tile_pool(name="wp", bufs=2))
            actp = ctx.enter_context(tc.tile_pool(name="actp", bufs=1))
            mixp = ctx.enter_context(tc.tile_pool(name="mixp", bufs=3))
            chp = ctx.enter_context(tc.tile_pool(name="chp", bufs=1))
            sm = ctx.enter_context(tc.tile_pool(name="sm", bufs=2))
            tmpp = ctx.enter_context(tc.tile_pool(name="tmpp", bufs=2))
            ps512 = ctx.enter_context(tc.tile_pool(name="ps512", bufs=5, space="PSUM"))
            pss = ctx.enter_context(tc.tile_pool(name="pss", bufs=3, space="PSUM"))

            # -------- gather the sharded weights over NeuronLink --------
            G = {}
            rg = [list(range(NCORES))]
            merge = {
                "Wr": "l p ct c -> (l p ct) c", "Wk": "l p ct c -> (l p ct) c",
                "Wv": "l p ct c -> (l p ct) c", "Wg": "l p ct c -> (l p ct) c",
                "Wo": "l p ct c -> (l p ct) c", "Wcr": "l p ct c -> (l p ct) c",
                "Wck": "l p ct c -> (l p ct) c", "Wcv": "l p ct c -> (l p ct) c",
                "wteT": "p ct v -> (p ct) v",
            }
            for name, (nat, (rows, cols)) in SHARD_SPECS.items():
                bounce = dpool.tile([rows // NCORES, cols], bf16)
                nc.gpsimd.dma_start(bounce[:], dram[name + "_sh"].ap())
                full = dpool.tile(list(nat), bf16, addr_space="Shared")
                nc.gpsimd.collective_compute(
                    "AllGather", mybir.AluOpType.bypass,
                    replica_groups=rg,
                    ins=[bounce[:]],
                    outs=[full.rearrange(merge[name])],
                )
                G[name] = full

            x_res = pers.tile([128, CT, T], f32)
            x0_sb = actp.tile([128, CT, T], bf16, tag="xln", bufs=1)
            nc.sync.dma_start(x0_sb[:], dram["x0"].ap())
            for ct in range(CT):
                nc.vector.tensor_copy(x_res[:, ct, :], x0_sb[:, ct, :])
            S_f = pers.tile([128, H, HD], f32)
            S_b = pers.tile([128, H, HD], bf16)
            ones_col = pers.tile([128, 1], f32)
            nc.gpsimd.memset(ones_col[:], 1.0)
            ones_row = pers.tile([1, 128], f32)
            nc.gpsimd.memset(ones_row[:], 1.0)
            ones_row_h = pers.tile([1, 128], bf16)
            nc.gpsimd.memset(ones_row_h[:], 1.0)
            blka = pers.tile([128, CT, H], bf16)
            nc.sync.dma_start(blka[:], dram["blk_a"].ap())
            blkaf = pers.tile([128, CT, H], f32)
            nc.sync.dma_start(blkaf[:], dram["blk_af"].ap())
            blkb = pers.tile([12, CT, 128], f32)
            nc.sync.dma_start(blkb[:], dram["blk_b"].ap())
            eps_ln = pers.tile([128, 1], f32)
            nc.gpsimd.memset(eps_ln[:], LN_EPS)
            eps_gn = pers.tile([128, 1], f32)
            nc.gpsimd.memset(eps_gn[:], GN_EPS)
            emat_t = pers.tile([128, 2, Q], bf16)
            nc.sync.dma_start(emat_t[:], dram["emat"].ap())
            mtri_t = pers.tile([128, 2, Q], bf16)
            nc.sync.dma_start(mtri_t[:], dram["mtri"].ap())
            meye_t = pers.tile([128, 2, Q], bf16)
            nc.sync.dma_start(meye_t[:], dram["meye"].ap())
            irow_t = pers.tile([128, Q], bf16)
            nc.sync.dma_start(irow_t[:], dram["irow"].ap())

            def layernorm(src):
                xln = actp.tile([128, CT, T], bf16, tag="xln", bufs=1)
                for tch in range(2):
                    tsl = slice(tch * 512, (tch + 1) * 512)
                    mu_ps = pss.tile([128, 512], f32, tag="pss")
                    m2_ps = pss.tile([128, 512], f32, tag="pss")
                    for ct in range(CT):
                        nc.tensor.matmul(mu_ps[0:1, :], ones_col[:], src[:, ct, tsl],
                                         start=(ct == 0), stop=(ct == CT - 1))
                    for ct in range(CT):
                        sq = tmpp.tile([128, 512], f32, tag="lnsq", bufs=1)
                        nc.vector.tensor_mul(sq[:], src[:, ct, tsl], src[:, ct, tsl])
                        nc.tensor.matmul(m2_ps[0:1, :], ones_col[:], sq[:],
                                         start=(ct == 0), stop=(ct == CT - 1))
                    mu_row = sm.tile([1, 512], f32, tag="rows", bufs=3)
                    nc.scalar.activation(mu_row[:], mu_ps[0:1, :], AF.Copy,
                                         scale=1.0 / C)
                    mu2_row = sm.tile([1, 512], f32, tag="rows", bufs=3)
                    nc.vector.tensor_mul(mu2_row[:], mu_row[:], mu_row[:])
                    var_row = sm.tile([1, 512], f32, tag="rows", bufs=3)
                    nc.vector.scalar_tensor_tensor(
                        var_row[:], m2_ps[0:1, :], 1.0 / C, mu2_row[:],
                        OP.mult, OP.subtract)
                    std_row = sm.tile([1, 512], f32, tag="rows", bufs=3)
                    nc.scalar.activation(std_row[:], var_row[:], AF.Sqrt,
                                         bias=eps_ln[0:1, :])
                    rstd_row = sm.tile([1, 512], f32, tag="rows", bufs=3)
                    nc.vector.reciprocal(rstd_row[:], std_row[:])
                    MU = ps512.tile([128, 512], f32, tag="ps512")
                    RSTD = ps512.tile([128, 512], f32, tag="ps512")
                    nc.tensor.matmul(MU[:], ones_row[:], mu_row[:],
                                     start=True, stop=True)
                    nc.tensor.matmul(RSTD[:], ones_row[:], rstd_row[:],
                                     start=True, stop=True)
                    for ct in range(CT):
                        t = tmpp.tile([128, 512], f32, tag="lnsq", bufs=1)
                        nc.vector.tensor_sub(t[:], src[:, ct, tsl], MU[:])
                        nc.vector.tensor_mul(xln[:, ct, tsl], t[:], RSTD[:])
                return xln

            def mix(xln, maa_t, kind, tch):
                """m = xln*(1-maa) + shift(xln)*maa for tokens [tch*512, +512)"""
                m = mixp.tile([128, CT, 512], bf16, tag="mix")
                lo = tch * 512
                for ct in range(CT):
                    nc.vector.tensor_scalar_mul(
                        m[:, ct, :], xln[:, ct, lo:lo + 512],
                        maa_t[:, ct, 6 + kind:7 + kind])
                    if tch == 0:
                        nc.vector.scalar_tensor_tensor(
                            m[:, ct, 1:512], xln[:, ct, 0:511],
                            maa_t[:, ct, kind:kind + 1], m[:, ct, 1:512],
                            OP.mult, OP.add)
                    else:
                        nc.vector.scalar_tensor_tensor(
                            m[:, ct, :], xln[:, ct, lo - 1:lo + 511],
                            maa_t[:, ct, kind:kind + 1], m[:, ct, :],
                            OP.mult, OP.add)
                return m

            def load_w(name, l, fsl=None):
                t = wp.tile([128, CT, C], bf16, tag="wcc")
                if fsl is None:
                    nc.sync.dma_start(t[:], G[name][l])
                else:
                    nc.sync.dma_start(t[:], G[name][l, :, :, fsl])
                return t

            # ============== layers ==============
            for l in range(n_layers):
                maa_t = lcon.tile([128, CT, 12], f32, tag="maa")
                nc.sync.dma_start(maa_t[:], dram["maa"].ap()[l])
                bias_t = lcon.tile([128, CT, 8], f32, tag="bias")
                nc.sync.dma_start(bias_t[:], dram["bias_cm"].ap()[l])
                bckt = lcon.tile([128, HT], f32, tag="bck")
                nc.sync.dma_start(bckt[:], dram["bck_t"].ap()[l])
                lnwh_t = lcon.tile([128, H], f32, tag="lnwh")
                nc.sync.dma_start(lnwh_t[:], dram["lnw_h"].ap()[l])
                uh_t = lcon.tile([128, H], f32, tag="uh")
                nc.sync.dma_start(uh_t[:], dram["u_h"].ap()[l])
                lnwct_t = lcon.tile([128, CT], f32, tag="lnwct")
                nc.sync.dma_start(lnwct_t[:], dram["lnw_ct"].ap()[l])
                # decay tables: wmT[p,h,jt,i] = exp(min(lnw_h*(i-j-1), 0))
                #   * [i>j] + u_h*[i==j]  (j = jt*128+p);  wbq = exp(lnw*i)
                wmT = lcon.tile([128, H, 2, Q], bf16, tag="wmT")
                for h in range(H):
                    for jt in range(2):
                        es = tmpp.tile([128, Q], f32, tag="wmes", bufs=1)
                        nc.vector.tensor_scalar(
                            es[:], emat_t[:, jt, :], lnwh_t[:, h:h + 1], 0.0,
                            OP.mult, OP.min)
                        pe = tmpp.tile([128, Q], bf16, tag="wmpe", bufs=1)
                        nc.scalar.activation(pe[:], es[:], AF.Exp)
                        pm = tmpp.tile([128, Q], bf16, tag="wmpm", bufs=1)
                        nc.vector.tensor_mul(pm[:], pe[:], mtri_t[:, jt, :])
                        nc.vector.scalar_tensor_tensor(
                            wmT[:, h, jt, :], meye_t[:, jt, :],
                            uh_t[:, h:h + 1], pm[:], OP.mult, OP.add)
                wbq = lcon.tile([128, CT, Q], bf16, tag="wbq")
                for ct in range(CT):
                    es = tmpp.tile([128, Q], f32, tag="wmes", bufs=1)
                    nc.vector.tensor_scalar(
                        es[:], irow_t[:], lnwct_t[:, ct:ct + 1], 0.0,
                        OP.mult, OP.min)
                    nc.scalar.activation(wbq[:, ct, :], es[:], AF.Exp)
                wkc = lcon.tile([128, H * 2], f32, tag="wk")
                nc.sync.dma_start(wkc[:], dram["wk_col"].ap()[l])
                wsbc = lcon.tile([128, H], f32, tag="ws")
                nc.sync.dma_start(wsbc[:], dram["ws_bc"].ap()[l])
                brow0 = lcon.tile([1, C], bf16, tag="brow0")
                nc.sync.dma_start(brow0[:], dram["bias_rows"].ap()[l, 0:1, :])
                brow1 = lcon.tile([1, C], bf16, tag="brow1")
                nc.sync.dma_start(brow1[:], dram["bias_rows"].ap()[l, 1:2, :])

                # ---- tmix ----
                xln = layernorm(x_res)
                kC = actp.tile([128, CT, T], bf16, tag="kh", bufs=1)
                kT = actp.tile([128, TT, C], bf16, tag="kT", bufs=1)
                vT = actp.tile([128, TT, C], bf16, tag="vT", bufs=1)
                r_sb = actp.tile([128, CT, T], bf16, tag="r", bufs=1)
                gC = actp.tile([128, CT, T], bf16, tag="gx", bufs=1)

                def proj_cm_dst(mx, wt, dst, func, bkind):
                    for mt in range(CT):
                        for tch in range(2):
                            tsl = slice(tch * 512, (tch + 1) * 512)
                            ps = ps512.tile([128, 512], f32, tag="ps512")
                            for kt in range(CT):
                                nc.tensor.matmul(
                                    ps[:], wt[:, kt, mt * 128:(mt + 1) * 128],
                                    mx[tch][:, kt, :],
                                    start=(kt == 0), stop=(kt == CT - 1))
                            if func == "silu":
                                sg = tmpp.tile([128, 512], bf16, tag="sg", bufs=1)
                                nc.scalar.activation(sg[:], ps[:], AF.Sigmoid,
                                                     bias=bias_t[:, mt, bkind:bkind + 1])
                                nc.vector.scalar_tensor_tensor(
                                    dst[:, mt, tsl], ps[:],
                                    bias_t[:, mt, bkind:bkind + 1], sg[:],
                                    OP.add, OP.mult)
                            else:
                                nc.scalar.activation(dst[:, mt, tsl], ps[:], func,
                                                     bias=bias_t[:, mt, bkind:bkind + 1])

                def proj_tm_dst(mx, wt, dst, brow):
                    for tch in range(2):
                        for tt4 in range(4):
                            tt = tch * 4 + tt4
                            for nch in range(2):
                                nsl = slice(nch * 384, (nch + 1) * 384)
                                ps = ps512.tile([128, 512], f32, tag="ps512")
                                for kt in range(CT):
                                    nc.tensor.matmul(
                                        ps[:, 0:384],
                                        mx[tch][:, kt, tt4 * 128:(tt4 + 1) * 128],
                                        wt[:, kt, nsl], start=(kt == 0), stop=False)
                                nc.tensor.matmul(ps[:, 0:384], ones_row_h[:],
                                                 brow[0:1, nsl], start=False,
                                                 stop=True)
                                nc.scalar.activation(dst[:, tt, nsl], ps[:, 0:384],
                                                     AF.Copy)

                wk_t = load_w("Wk", l)
                xk = [mix(xln, maa_t, 0, 0), mix(xln, maa_t, 0, 1)]
                proj_cm_dst(xk, wk_t, kC, AF.Identity, 1)
                proj_tm_dst(xk, wk_t, kT, brow0)
                wv_t = load_w("Wv", l)
                xv = [mix(xln, maa_t, 1, 0), mix(xln, maa_t, 1, 1)]
                proj_tm_dst(xv, wv_t, vT, brow1)
                wr_t = load_w("Wr", l)
                xr = [mix(xln, maa_t, 2, 0), mix(xln, maa_t, 2, 1)]
                proj_cm_dst(xr, wr_t, r_sb, AF.Identity, 0)
                wg_t = load_w("Wg", l)
                xg = [mix(xln, maa_t, 3, 0), mix(xln, maa_t, 3, 1)]
                proj_cm_dst(xg, wg_t, gC, "silu", 6)

                # ---- chunked attention ----
                wo_t = load_w("Wo", l)
                for ch in range(NCH):
                    csl = slice(ch * Q, (ch + 1) * Q)
                    yT = chp.tile([128, CT, Q], bf16, tag="yT")
                    for ct in range(CT):
                        if ch > 0:
                            rwf = tmpp.tile([128, Q], bf16, tag="rwf", bufs=2)
                            nc.vector.tensor_mul(rwf[:], r_sb[:, ct, csl],
                                                 wbq[:, ct, :])
                        y_ps = ps512.tile([128, 512], f32, tag="ps512")
                        for hh in range(2):
                            h = 2 * ct + hh
                            hp = hh * 64
                            a_ps = ps512.tile([128, 512], f32, tag="ps512")
                            for jt in range(2):
                                nc.tensor.matmul(
                                    a_ps[:, jt * Q:(jt + 1) * Q],
                                    kC[hp:hp + 64, ct,
                                       (2 * ch + jt) * 128:(2 * ch + jt + 1) * 128],
                                    r_sb[hp:hp + 64, ct, csl],
                                    start=True, stop=True)
                            a_sb = tmpp.tile([128, 512], bf16, tag="attT", bufs=2)
                            for jt in range(2):
                                nc.vector.tensor_mul(
                                    a_sb[:, jt * Q:(jt + 1) * Q],
                                    a_ps[:, jt * Q:(jt + 1) * Q], wmT[:, h, jt, :])
                            ysl = y_ps[hp:hp + 64, 0:Q]
                            for jt in range(2):
                                nc.tensor.matmul(
                                    ysl, vT[:, 2 * ch + jt, h * 64:(h + 1) * 64],
                                    a_sb[:, jt * Q:(jt + 1) * Q],
                                    start=(jt == 0),
                                    stop=(jt == 1 and ch == 0))
                            if ch > 0:
                                nc.tensor.matmul(ysl, S_b[hp:hp + 64, h, :],
                                                 rwf[hp:hp + 64, :],
                                                 start=False, stop=True)
                            kw = tmpp.tile([128, 2, HD], bf16, tag="kw", bufs=1)
                            for jt in range(2):
                                nc.vector.tensor_scalar_mul(
                                    kw[:, jt, :],
                                    kT[:, 2 * ch + jt, h * 64:(h + 1) * 64],
                                    wkc[:, h * 2 + jt:h * 2 + jt + 1])
                            c_ps = pss.tile([128, 512], f32, tag="pss")
                            for jt in range(2):
                                nc.tensor.matmul(
                                    c_ps[hp:hp + 64, 0:64], kw[:, jt, :],
                                    vT[:, 2 * ch + jt, h * 64:(h + 1) * 64],
                                    start=(jt == 0), stop=(jt == 1))
                            if ch == 0:
                                nc.vector.tensor_copy(S_f[hp:hp + 64, h, :],
                                                      c_ps[hp:hp + 64, 0:64])
                            else:
                                nc.vector.scalar_tensor_tensor(
                                    S_f[hp:hp + 64, h, :], S_f[hp:hp + 64, h, :],
                                    wsbc[hp:hp + 64, h:h + 1],
                                    c_ps[hp:hp + 64, 0:64], OP.mult, OP.add)
                            if ch < NCH - 1:
                                nc.vector.tensor_copy(S_b[hp:hp + 64, h, :],
                                                      S_f[hp:hp + 64, h, :])
                        nc.scalar.activation(yT[:, ct, :], y_ps[:, 0:Q], AF.Copy)

                    # GroupNorm: two-pass (center in place, then sum of squares)
                    mu_ps = pss.tile([128, 512], f32, tag="pss")
                    for ct in range(CT):
                        nc.tensor.matmul(mu_ps[0:H, 0:Q], blka[:, ct, :], yT[:, ct, :],
                                         start=(ct == 0), stop=(ct == CT - 1))
                    mu_sb = sm.tile([12, Q], f32, tag="gvar", bufs=3)
                    nc.scalar.activation(mu_sb[:], mu_ps[0:H, 0:Q], AF.Copy,
                                         scale=1.0 / HD)
                    for ct in range(CT):
                        MUb = ps512.tile([128, 512], f32, tag="ps512")
                        nc.tensor.matmul(MUb[:, 0:Q], blkb[:, ct, :], mu_sb[:],
                                         start=True, stop=True)
                        nc.vector.tensor_sub(yT[:, ct, :], yT[:, ct, :], MUb[:, 0:Q])
                    m2_ps = pss.tile([128, 512], f32, tag="pss")
                    for ct in range(CT):
                        sq = tmpp.tile([128, Q], f32, tag="gnsq", bufs=2)
                        nc.vector.tensor_mul(sq[:], yT[:, ct, :], yT[:, ct, :])
                        nc.tensor.matmul(m2_ps[0:H, 0:Q], blkaf[:, ct, :], sq[:],
                                         start=(ct == 0), stop=(ct == CT - 1))
                    var_sb = sm.tile([12, Q], f32, tag="gvar", bufs=3)
                    nc.scalar.activation(var_sb[:], m2_ps[0:H, 0:Q], AF.Copy,
                                         scale=1.0 / HD)
                    std_sb = sm.tile([12, Q], f32, tag="gvar", bufs=3)
                    nc.scalar.activation(std_sb[:], var_sb[:], AF.Sqrt,
                                         bias=eps_gn[0:12, :])
                    rstd_sb = sm.tile([12, Q], f32, tag="gvar", bufs=3)
                    nc.vector.reciprocal(rstd_sb[:], std_sb[:])
                    prod = chp.tile([128, CT, Q], bf16, tag="prod")
                    for ct in range(CT):
                        RSb = ps512.tile([128, 512], f32, tag="ps512")
                        nc.tensor.matmul(RSb[:, 0:Q], blkb[:, ct, :], rstd_sb[:],
                                         start=True, stop=True)
                        t2 = tmpp.tile([128, Q], f32, tag="gnt", bufs=2)
                        nc.vector.tensor_mul(t2[:], yT[:, ct, :], RSb[:, 0:Q])
                        nc.vector.scalar_tensor_tensor(
                            prod[:, ct, :], t2[:], bias_t[:, ct, 5:6],
                            gC[:, ct, csl], OP.add, OP.mult)
                    for mt in range(CT):
                        ps = ps512.tile([128, 512], f32, tag="ps512")
                        for kt in range(CT):
                            nc.tensor.matmul(
                                ps[:, 0:Q], wo_t[:, kt, mt * 128:(mt + 1) * 128],
                                prod[:, kt, :], start=(kt == 0), stop=(kt == CT - 1))
                        nc.vector.scalar_tensor_tensor(
                            x_res[:, mt, csl], ps[:, 0:Q], bias_t[:, mt, 2:3],
                            x_res[:, mt, csl], OP.add, OP.add)

                # ---- cmix ----
                xln2 = layernorm(x_res)
                wcr_t = load_w("Wcr", l)
                xr2 = [mix(xln2, maa_t, 5, 0), mix(xln2, maa_t, 5, 1)]
                gate = actp.tile([128, CT, T], bf16, tag="gx", bufs=1)
                for mt in range(CT):
                    for tch in range(2):
                        tsl = slice(tch * 512, (tch + 1) * 512)
                        ps = ps512.tile([128, 512], f32, tag="ps512")
                        for kt in range(CT):
                            nc.tensor.matmul(
                                ps[:], wcr_t[:, kt, mt * 128:(mt + 1) * 128],
                                xr2[tch][:, kt, :],
                                start=(kt == 0), stop=(kt == CT - 1))
                        nc.scalar.activation(gate[:, mt, tsl], ps[:], AF.Sigmoid,
                                             bias=bias_t[:, mt, 4:5])
                xk2 = [mix(xln2, maa_t, 4, 0), mix(xln2, maa_t, 4, 1)]
                for tch in range(2):
                    tsl = slice(tch * 512, (tch + 1) * 512)
                    h2 = actp.tile([128, HT, 512], bf16, tag="kh", bufs=1)
                    for third in range(3):
                        wck_t = load_w("Wck", l, slice(third * C, (third + 1) * C))
                        for mt6 in range(6):
                            gmt = third * 6 + mt6
                            ps = ps512.tile([128, 512], f32, tag="ps512")
                            for kt in range(CT):
                                nc.tensor.matmul(
                                    ps[:], wck_t[:, kt, mt6 * 128:(mt6 + 1) * 128],
                                    xk2[tch][:, kt, :],
                                    start=(kt == 0), stop=(kt == CT - 1))
                            hr = tmpp.tile([128, 512], bf16, tag="hrelu", bufs=2)
                            nc.vector.tensor_scalar(
                                hr[:], ps[:], bckt[:, gmt:gmt + 1], 0.0,
                                OP.add, OP.max)
                            nc.vector.tensor_mul(h2[:, gmt, :], hr[:], hr[:])
                    for third in range(3):
                        wcv_t = wp.tile([128, HT, Q], bf16, tag="wcc")
                        nc.sync.dma_start(
                            wcv_t[:],
                            G["Wcv"][l, :, :, third * Q:(third + 1) * Q])
                        for mt2 in range(2):
                            gmt = third * 2 + mt2
                            ps = ps512.tile([128, 512], f32, tag="ps512")
                            for kt in range(HT):
                                nc.tensor.matmul(
                                    ps[:], wcv_t[:, kt, mt2 * 128:(mt2 + 1) * 128],
                                    h2[:, kt, :], start=(kt == 0), stop=(kt == HT - 1))
                            t = tmpp.tile([128, 512], f32, tag="cvt", bufs=1)
                            nc.vector.scalar_tensor_tensor(
                                t[:], ps[:], bias_t[:, gmt, 3:4], gate[:, gmt, tsl],
                                OP.add, OP.mult)
                            nc.vector.tensor_add(x_res[:, gmt, tsl],
                                                 x_res[:, gmt, tsl], t[:])

            # ============== head ==============
            sq_sb = sm.tile([128, CT, 2], f32, tag="hsq")
            for ct in range(CT):
                nc.vector.tensor_mul(sq_sb[:, ct, 1:2], x_res[:, ct, T - 1:T],
                                     x_res[:, ct, T - 1:T])
                nc.vector.tensor_copy(sq_sb[:, ct, 0:1], x_res[:, ct, T - 1:T])
            mu_ps = pss.tile([128, 512], f32, tag="pss")
            for ct in range(CT):
                nc.tensor.matmul(mu_ps[0:1, 0:2], ones_col[:], sq_sb[:, ct, :],
                                 start=(ct == 0), stop=(ct == CT - 1))
            st_row = sm.tile([1, 2], f32, tag="hrow", bufs=4)
            nc.scalar.activation(st_row[:], mu_ps[0:1, 0:2], AF.Copy, scale=1.0 / C)
            mu2_row = sm.tile([1, 1], f32, tag="hrow", bufs=4)
            nc.vector.tensor_mul(mu2_row[:], st_row[:, 0:1], st_row[:, 0:1])
            var_row = sm.tile([1, 1], f32, tag="hrow", bufs=4)
            nc.vector.tensor_sub(var_row[:], st_row[:, 1:2], mu2_row[:])
            stdh_row = sm.tile([1, 1], f32, tag="hrow", bufs=4)
            nc.scalar.activation(stdh_row[:], var_row[:], AF.Sqrt,
                                 bias=eps_ln[0:1, :])
            rstd_row = sm.tile([1, 1], f32, tag="hrow", bufs=4)
            nc.vector.reciprocal(rstd_row[:], stdh_row[:])
            MU128 = sm.tile([128, 1], f32, tag="hb")
            RSTD128 = sm.tile([128, 1], f32, tag="hb")
            nc.gpsimd.partition_broadcast(MU128[:], st_row[:, 0:1])
            nc.gpsimd.partition_broadcast(RSTD128[:], rstd_row[:])
            xl = sm.tile([128, CT], bf16, tag="xl")
            for ct in range(CT):
                nc.vector.scalar_tensor_tensor(
                    xl[:, ct:ct + 1], x_res[:, ct, T - 1:T], MU128[:],
                    RSTD128[:], OP.subtract, OP.mult)
            nv = (V + 511) // 512
            for nt in range(nv):
                nsz = min(512, V - nt * 512)
                ps = pss.tile([128, 512], f32, tag="pss")
                for kt in range(CT):
                    wv_sb = tmpp.tile([128, 512], bf16, tag="hw", bufs=2)
                    nc.sync.dma_start(
                        wv_sb[:, 0:nsz],
                        G["wteT"][:, kt, nt * 512:nt * 512 + nsz])
                    nc.tensor.matmul(ps[0:1, 0:nsz], xl[:, kt:kt + 1],
                                     wv_sb[:, 0:nsz],
                                     start=(kt == 0), stop=(kt == CT - 1))
                ot = sm.tile([1, 512], f32, tag="hout")
                nc.scalar.activation(ot[:, 0:nsz], ps[0:1, 0:nsz], AF.Copy)
                nc.sync.dma_start(out_logits.ap()[:, nt * 512:nt * 512 + nsz],
                                  ot[:, 0:nsz])

    nc.compile()
    return nc


def _get_program(n_layers=L):
    if n_layers not in _PROG_CACHE:
        _PROG_CACHE[n_layers] = _build_program(n_layers)
    return _PROG_CACHE[n_layers]


def kernel(**inputs):
    from concourse.bass_utils import run_bass_kernel_spmd

    in_maps, lbias = _host_precompute(inputs)
    nc = _get_program(L)
    res = run_bass_kernel_spmd(nc, in_maps, core_ids=list(range(NCORES)),
                               trace=False)
    out = np.zeros((B, 1, V), np.float32)
    for b in range(B):
        out[b, 0, :] = res.results[b]["logits"][0]
    out += lbias[None, None, :]
    return out


# revision 32
# speedup vs baseline: 1.0761x; 1.0761x over previous
# nn_GPT_64347200029289 — RWKV6-style dense transformer on 8 TRN2 NeuronCores.
# B=4, T=1024, C=768, H=12 heads (headdim 64), L=12 layers, V=50304.
# Output: last-position logits [B, 1, V].
#
# Sharding: the host→device tunnel is the bottleneck (~75 MB/s effective), so
# all large weight tensors are shipped 8-way sharded (each core receives a
# distinct 1/8 slice) and reassembled on-device with AllGather collectives
# over NeuronLink. After the gather every core holds the full weights in
# internal DRAM; core c then runs the full 12-layer body for batch c%4 and
# computes full-vocab logits (cores 4-7 duplicate 0-3; host keeps cores 0-3).
#
# Layout: residual kept C-major ([C-tile=128 partitions, T free], fp32).
# All matmuls bf16 with fp32 PSUM accumulation. LayerNorm/GroupNorm affine
# params are folded into adjacent projection weights on the host. LN stats are
# partition reductions via ones-matmuls; GroupNorm stats use block-diagonal
# ones-matmuls so y stays C-major (no transposes anywhere). Attention is the
# chunked RWKV scan (Q=256, 4 chunks) with host-precomputed decay tables.

import sys
import numpy as np

sys.path.insert(0, "/opt/trn_rl_repo")

import ml_dtypes

# Persistent jax compilation cache: run_bass_kernel_spmd builds a fresh
# jax.jit per call, which otherwise re-runs the ~5s neuronxcc compile of the
# (unchanged) NEFF on every call.
try:
    import jax
    jax.config.update("jax_compilation_cache_dir", "/tmp/jaxcache")
    jax.config.update("jax_persistent_cache_min_compile_time_secs", 0.0)
    jax.config.update("jax_persistent_cache_min_entry_size_bytes", 0)
except Exception:
    pass

C, H, L, V, BLK = 768, 12, 12, 50304, 1024
HD = C // H                  # 64
B, T, Q = 4, 1024, 256
NCH = T // Q                 # 4 chunks
CT = C // 128                # 6
TT = T // 128                # 8
HT = (3 * C) // 128          # 18
GN_EPS = 1e-5 * 64
LN_EPS = 1e-5
NCORES = 8

BF = np.float16

# name -> (natural shape, 2D collective shape [rows, cols]); rows % 8 == 0,
# rank-r shard is rows/8 consecutive rows. Gathered tensor layout == natural.
SHARD_SPECS = {
    "Wr":   ((L, 128, CT, C), (L * 128 * CT, C)),
    "Wk":   ((L, 128, CT, C), (L * 128 * CT, C)),
    "Wv":   ((L, 128, CT, C), (L * 128 * CT, C)),
    "Wg":   ((L, 128, CT, C), (L * 128 * CT, C)),
    "Wo":   ((L, 128, CT, C), (L * 128 * CT, C)),
    "Wcr":  ((L, 128, CT, C), (L * 128 * CT, C)),
    "Wck":  ((L, 128, CT, 3 * C), (L * 128 * CT, 3 * C)),
    "Wcv":  ((L, 128, HT, C), (L * 128 * HT, C)),
    "wteT": ((128, CT, V), (128 * CT, V)),
}

# Small replicated tables, packed into one f32 + one fp16 blob that are
# 8-way sharded and AllGathered like the big weights (order matters: the
# program reconstructs views at these offsets).
F32_SMALLS = [
    ("maa", (L, 128, CT, 12)), ("bias_cm", (L, 128, CT, 8)),
    ("bck_t", (L, 128, HT)), ("wk_col", (L, 128, H * 2)),
    ("ws_bc", (L, 128, H)), ("blk_af", (128, CT, H)),
    ("blk_b", (12, CT, 128)), ("lnw_h", (L, 128, H)),
    ("u_h", (L, 128, H)), ("lnw_ct", (L, 128, CT)),
]
H16_SMALLS = [
    ("bias_rows", (L, 2, C)), ("blk_a", (128, CT, H)),
    ("emat", (128, 2, Q)), ("mtri", (128, 2, Q)), ("meye", (128, 2, Q)),
    ("irow", (128, Q)),
]


def _blob_pack(smalls, specs, dtype):
    parts = [np.ascontiguousarray(smalls[n], dtype).ravel() for n, s in specs]
    flat = np.concatenate(parts)
    pad = (-flat.size) % (NCORES * 128)
    if pad:
        flat = np.concatenate([flat, np.zeros(pad, dtype)])
    return flat.reshape(-1, 128)


def _host_precompute(inputs):
    f = lambda k: np.asarray(inputs[k], np.float32)
    idx = np.asarray(inputs["idx"])
    wte, wpe = f("wte"), f("wpe")
    ln1_w, ln1_b = f("ln1_w"), f("ln1_b")
    ln2_w, ln2_b = f("ln2_w"), f("ln2_b")
    gn_w, gn_b = f("gn_w"), f("gn_b")
    lnf_w, lnf_b = f("lnf_w"), f("lnf_b")
    Wr, Wk, Wv, Wg, Wo = f("Wr"), f("Wk"), f("Wv"), f("Wg"), f("Wo")
    Wck, Wcv, Wcr = f("Wck"), f("Wcv"), f("Wcr")
    br, bk, bv, bg, bo = f("br"), f("bk"), f("bv"), f("bg"), f("bo")
    bck, bcv, bcr = f("bck"), f("bcv"), f("bcr")
    maa_tk, maa_tv = f("maa_tk"), f("maa_tv")
    maa_tr, maa_tg = f("maa_tr"), f("maa_tg")
    cmaa_k, cmaa_r = f("cmaa_k"), f("cmaa_r")
    tdecay, tfaaaa = f("tdecay"), f("tfaaaa")

    def fold(W, lw, lb, bproj):
        We = lw[:, :, None] * W
        be = bproj + np.einsum("lc,lco->lo", lb, W)
        return We, be

    Wr_e, br_e = fold(Wr, ln1_w, ln1_b, br)
    Wk_e, bk_e = fold(Wk, ln1_w, ln1_b, bk)
    Wv_e, bv_e = fold(Wv, ln1_w, ln1_b, bv)
    Wg_e, bg_e = fold(Wg, ln1_w, ln1_b, bg)
    Wck_e, bck_e = fold(Wck, ln2_w, ln2_b, bck)
    Wcr_e, bcr_e = fold(Wcr, ln2_w, ln2_b, bcr)
    Wo_e = gn_w[:, :, None] * Wo
    Wcv_e = Wcv
    bo_e, bcv_e = bo, bcv

    w = np.exp(-np.exp(tdecay)).astype(np.float64)       # [L,H]
    ii = np.arange(Q)
    wk_ = (w[:, :, None] ** (Q - 1 - ii)[None, None, :]).astype(np.float32)
    ws_ = (w ** Q).astype(np.float32)

    wteT_e = lnf_w[:, None] * wte.T                      # [C,V]
    lbias = lnf_b @ wte.T                                # [V]
    x0 = wte[idx] + wpe[:T]                              # [B,T,C]

    def cm(M):  # [Cin,F] -> [128, Cin//128, F]
        Cin, F2 = M.shape
        return np.ascontiguousarray(M.reshape(Cin // 128, 128, F2).transpose(1, 0, 2))

    big = {}
    for name, We in (("Wr", Wr_e), ("Wk", Wk_e), ("Wv", Wv_e), ("Wg", Wg_e),
                     ("Wo", Wo_e), ("Wcr", Wcr_e), ("Wck", Wck_e), ("Wcv", Wcv_e)):
        big[name] = np.stack([cm(We[l]) for l in range(L)]).astype(BF)
    big["wteT"] = cm(wteT_e).astype(BF)                  # [128, CT, V]

    # wmT/wbq decay tables are generated on-device from lnw = ln(w) =
    # -exp(tdecay): wmT[l,p,h,jt,i] = exp(min(lnw*(i-j-1),0))*[i>j] + u*[i==j]
    # with j = jt*128+p; wbq[l,p,ct,i] = exp(lnw_head(ct,p) * i).
    lnw = (-np.exp(tdecay)).astype(np.float32)           # [L,H]
    lnw_h = np.ascontiguousarray(
        np.broadcast_to(lnw[:, None, :], (L, 128, H))).astype(np.float32)
    u_h = np.ascontiguousarray(
        np.broadcast_to(tfaaaa[:, None, :], (L, 128, H))).astype(np.float32)
    lnw_ct = np.zeros((L, 128, CT), np.float32)
    for ct in range(CT):
        lnw_ct[:, 0:64, ct] = lnw[:, 2 * ct, None]
        lnw_ct[:, 64:128, ct] = lnw[:, 2 * ct + 1, None]
    ivec = np.arange(Q, dtype=np.float32)
    jvec = np.arange(128, dtype=np.float32)
    emat = np.zeros((128, 2, Q), np.float32)
    mtri = np.zeros((128, 2, Q), np.float32)
    meye = np.zeros((128, 2, Q), np.float32)
    for jt in range(2):
        jj = jt * 128 + jvec[:, None]
        emat[:, jt, :] = ivec[None, :] - jj - 1.0
        mtri[:, jt, :] = (ivec[None, :] > jj).astype(np.float32)
        meye[:, jt, :] = (ivec[None, :] == jj).astype(np.float32)
    irow = np.broadcast_to(ivec[None, :], (128, Q))

    wk_col = wk_.reshape(L, H, 2, 128).transpose(0, 3, 1, 2).reshape(L, 128, H * 2)
    ws_bc = np.zeros((L, 128, H), np.float32)
    ws_bc[:, 0:64, :] = ws_[:, None, :]
    ws_bc[:, 64:128, :] = ws_[:, None, :]

    # mix coefficients [L,128,CT,12]: kinds tk,tv,tr,tg,ck,cr then negated
    maa_all = np.stack([maa_tk, maa_tv, maa_tr,
                        maa_tg, cmaa_k, cmaa_r], axis=-1)   # [L,C,6]
    maa_all = np.concatenate([maa_all, 1.0 - maa_all], axis=-1)
    maa_pack = maa_all.reshape(L, CT, 128, 12).transpose(0, 2, 1, 3)

    # C-major per-partition biases [L,128,CT,8]: br,bkC,bo,bcv,bcr,gnb,bg,pad
    bias_cm = np.stack([br_e, bk_e, bo_e, bcv_e, bcr_e,
                        np.broadcast_to(gn_b, br_e.shape), bg_e,
                        np.zeros_like(br_e)], axis=-1)
    bias_cm = bias_cm.reshape(L, CT, 128, 8).transpose(0, 2, 1, 3)
    bck_t = bck_e.reshape(L, HT, 128).transpose(0, 2, 1)          # [L,128,HT]
    bias_rows = np.stack([bk_e, bv_e], axis=1)                    # [L,2,C]

    blk_a = np.zeros((128, CT, H), np.float32)
    blk_b = np.zeros((12, CT, 128), np.float32)
    for ct in range(CT):
        blk_a[0:64, ct, 2 * ct] = 1.0
        blk_a[64:128, ct, 2 * ct + 1] = 1.0
        blk_b[2 * ct, ct, 0:64] = 1.0
        blk_b[2 * ct + 1, ct, 64:128] = 1.0

    smalls = {
        "maa": maa_pack, "bias_cm": bias_cm, "bck_t": bck_t,
        "bias_rows": bias_rows, "wk_col": wk_col, "ws_bc": ws_bc,
        "blk_a": blk_a, "blk_af": blk_a, "blk_b": blk_b,
        "lnw_h": lnw_h, "u_h": u_h, "lnw_ct": lnw_ct,
        "emat": emat, "mtri": mtri, "meye": meye, "irow": irow,
    }
    fblob = _blob_pack(smalls, F32_SMALLS, np.float32)
    hblob = _blob_pack(smalls, H16_SMALLS, BF)
    frs, hrs = fblob.shape[0] // NCORES, hblob.shape[0] // NCORES
    common = {}

    # 1/8 row-shards of each big tensor (concat over ranks == natural layout)
    for name, (nat, two_d) in SHARD_SPECS.items():
        rows, cols = two_d
        arr = big[name].reshape(rows, cols)
        rs = rows // NCORES
        big[name] = [np.ascontiguousarray(arr[c * rs:(c + 1) * rs])
                     for c in range(NCORES)]

    x0cm = [np.ascontiguousarray(
        x0[b].T.reshape(CT, 128, T).transpose(1, 0, 2)).astype(BF)
        for b in range(4)]
    in_maps = []
    for c in range(NCORES):
        b = c % 4
        m = dict(common)
        for name in SHARD_SPECS:
            m[name + "_sh"] = big[name][c]
        m["fsm_sh"] = np.ascontiguousarray(fblob[c * frs:(c + 1) * frs])
        m["hsm_sh"] = np.ascontiguousarray(hblob[c * hrs:(c + 1) * hrs])
        m["x0"] = x0cm[b]
        in_maps.append(m)
    return in_maps, lbias


# ---------------------------------------------------------------------------

_PROG_CACHE = {}


def _build_program(n_layers=L):
    import concourse.bass as bass
    import concourse.tile as tile
    from concourse import mybir, bacc
    from contextlib import ExitStack

    f32 = mybir.dt.float32
    bf16 = mybir.dt.float16
    AF = mybir.ActivationFunctionType
    OP = mybir.AluOpType

    nc = bacc.Bacc("TRN2", target_bir_lowering=False, debug=False,
                   num_devices=NCORES)

    dram = {}
    def din(name, shape, dt=bf16):
        dram[name] = nc.dram_tensor(name, list(shape), dt, kind="ExternalInput")

    din("x0", (128, CT, T))
    for name, (nat, (rows, cols)) in SHARD_SPECS.items():
        din(name + "_sh", (rows // NCORES, cols))
    nf32 = sum(int(np.prod(s)) for _, s in F32_SMALLS)
    nh16 = sum(int(np.prod(s)) for _, s in H16_SMALLS)
    frows = -(-nf32 // 128 // NCORES) * NCORES
    hrows = -(-nh16 // 128 // NCORES) * NCORES
    din("fsm_sh", (frows // NCORES, 128), f32)
    din("hsm_sh", (hrows // NCORES, 128))
    out_logits = nc.dram_tensor("logits", [1, V], f32, kind="ExternalOutput")

    with tile.TileContext(nc) as tc:
        with ExitStack() as ctx:
            dpool = ctx.enter_context(tc.tile_pool(name="dpool", bufs=1,
                                                   space="DRAM"))
            pers = ctx.enter_context(tc.tile_pool(name="pers", bufs=1))
            lcon = ctx.enter_context(tc.tile_pool(name="lcon", bufs=1))
            wp = ctx.enter_context(tc.tile_pool(name="wp", bufs=2))
            actp = ctx.enter_context(tc.tile_pool(name="actp", bufs=1))
            mixp = ctx.enter_context(tc.tile_pool(name="mixp", bufs=3))
            chp = ctx.enter_context(tc.tile_pool(name="chp", bufs=1))
            sm = ctx.enter_context(tc.tile_pool(name="sm", bufs=2))
            tmpp = ctx.enter_context(tc.tile_pool(name="tmpp", bufs=2))
            ps512 = ctx.enter_context(tc.tile_pool(name="ps512", bufs=5, space="PSUM"))
            pss = ctx.enter_context(tc.tile_pool(name="pss", bufs=3, space="PSUM"))

            # -------- gather the sharded weights over NeuronLink --------
            G = {}
            rg = [list(range(NCORES))]
            merge = {
                "Wr": "l p ct c -> (l p ct) c", "Wk": "l p ct c -> (l p ct) c",
                "Wv": "l p ct c -> (l p ct) c", "Wg": "l p ct c -> (l p ct) c",
                "Wo": "l p ct c -> (l p ct) c", "Wcr": "l p ct c -> (l p ct) c",
                "Wck": "l p ct c -> (l p ct) c", "Wcv": "l p ct c -> (l p ct) c",
                "wteT": "p ct v -> (p ct) v",
            }
            for name, (nat, (rows, cols)) in SHARD_SPECS.items():
                bounce = dpool.tile([rows // NCORES, cols], bf16)
                nc.gpsimd.dma_start(bounce[:], dram[name + "_sh"].ap())
                full = dpool.tile(list(nat), bf16, addr_space="Shared")
                nc.gpsimd.collective_compute(
                    "AllGather", mybir.AluOpType.bypass,
                    replica_groups=rg,
                    ins=[bounce[:]],
                    outs=[full.rearrange(merge[name])],
                )
                G[name] = full

            def gather_blob(inp_name, n_rows, dt, specs):
                b_ = dpool.tile([n_rows // NCORES, 128], dt)
                nc.gpsimd.dma_start(b_[:], dram[inp_name].ap())
                fl = dpool.tile([n_rows, 128], dt, addr_space="Shared")
                nc.gpsimd.collective_compute(
                    "AllGather", mybir.AluOpType.bypass, replica_groups=rg,
                    ins=[b_[:]], outs=[fl[:]])
                flat = fl.rearrange("r c -> (r c)")
                views, off = {}, 0
                for nm, shape in specs:
                    n = int(np.prod(shape))
                    pat = ("(" + " ".join(f"d{i}" for i in range(len(shape)))
                           + ") -> " + " ".join(f"d{i}" for i in range(len(shape))))
                    views[nm] = flat[off:off + n].rearrange(
                        pat, **{f"d{i}": s for i, s in enumerate(shape)})
                    off += n
                return views
            VF = gather_blob("fsm_sh", frows, f32, F32_SMALLS)
            VH = gather_blob("hsm_sh", hrows, bf16, H16_SMALLS)

            x_res = pers.tile([128, CT, T], f32)
            x0_sb = actp.tile([128, CT, T], bf16, tag="xln", bufs=1)
            nc.sync.dma_start(x0_sb[:], dram["x0"].ap())
            for ct in range(CT):
                nc.vector.tensor_copy(x_res[:, ct, :], x0_sb[:, ct, :])
            S_f = pers.tile([128, H, HD], f32)
            S_b = pers.tile([128, H, HD], bf16)
            ones_col = pers.tile([128, 1], f32)
            nc.gpsimd.memset(ones_col[:], 1.0)
            ones_row = pers.tile([1, 128], f32)
            nc.gpsimd.memset(ones_row[:], 1.0)
            ones_row_h = pers.tile([1, 128], bf16)
            nc.gpsimd.memset(ones_row_h[:], 1.0)
            blka = pers.tile([128, CT, H], bf16)
            nc.sync.dma_start(blka[:], VH["blk_a"])
            blkaf = pers.tile([128, CT, H], f32)
            nc.sync.dma_start(blkaf[:], VF["blk_af"])
            blkb = pers.tile([12, CT, 128], f32)
            nc.sync.dma_start(blkb[:], VF["blk_b"])
            eps_ln = pers.tile([128, 1], f32)
            nc.gpsimd.memset(eps_ln[:], LN_EPS)
            eps_gn = pers.tile([128, 1], f32)
            nc.gpsimd.memset(eps_gn[:], GN_EPS)
            emat_t = pers.tile([128, 2, Q], bf16)
            nc.sync.dma_start(emat_t[:], VH["emat"])
            mtri_t = pers.tile([128, 2, Q], bf16)
            nc.sync.dma_start(mtri_t[:], VH["mtri"])
            meye_t = pers.tile([128, 2, Q], bf16)
            nc.sync.dma_start(meye_t[:], VH["meye"])
            irow_t = pers.tile([128, Q], bf16)
            nc.sync.dma_start(irow_t[:], VH["irow"])

            def layernorm(src):
                xln = actp.tile([128, CT, T], bf16, tag="xln", bufs=1)
                for tch in range(2):
                    tsl = slice(tch * 512, (tch + 1) * 512)
                    mu_ps = pss.tile([128, 512], f32, tag="pss")
                    m2_ps = pss.tile([128, 512], f32, tag="pss")
                    for ct in range(CT):
                        nc.tensor.matmul(mu_ps[0:1, :], ones_col[:], src[:, ct, tsl],
                                         start=(ct == 0), stop=(ct == CT - 1))
                    for ct in range(CT):
                        sq = tmpp.tile([128, 512], f32, tag="lnsq", bufs=1)
                        nc.vector.tensor_mul(sq[:], src[:, ct, tsl], src[:, ct, tsl])
                        nc.tensor.matmul(m2_ps[0:1, :], ones_col[:], sq[:],
                                         start=(ct == 0), stop=(ct == CT - 1))
                    mu_row = sm.tile([1, 512], f32, tag="rows", bufs=3)
                    nc.scalar.activation(mu_row[:], mu_ps[0:1, :], AF.Copy,
                                         scale=1.0 / C)
                    mu2_row = sm.tile([1, 512], f32, tag="rows", bufs=3)
                    nc.vector.tensor_mul(mu2_row[:], mu_row[:], mu_row[:])
                    var_row = sm.tile([1, 512], f32, tag="rows", bufs=3)
                    nc.vector.scalar_tensor_tensor(
                        var_row[:], m2_ps[0:1, :], 1.0 / C, mu2_row[:],
                        OP.mult, OP.subtract)
                    std_row = sm.tile([1, 512], f32, tag="rows", bufs=3)
                    nc.scalar.activation(std_row[:], var_row[:], AF.Sqrt,
                                         bias=eps_ln[0:1, :])
                    rstd_row = sm.tile([1, 512], f32, tag="rows", bufs=3)
                    nc.vector.reciprocal(rstd_row[:], std_row[:])
                    MU = ps512.tile([128, 512], f32, tag="ps512")
                    RSTD = ps512.tile([128, 512], f32, tag="ps512")
                    nc.tensor.matmul(MU[:], ones_row[:], mu_row[:],
                                     start=True, stop=True)
                    nc.tensor.matmul(RSTD[:], ones_row[:], rstd_row[:],
                                     start=True, stop=True)
                    for ct in range(CT):
                        t = tmpp.tile([128, 512], f32, tag="lnsq", bufs=1)
                        nc.vector.tensor_sub(t[:], src[:, ct, tsl], MU[:])
                        nc.vector.tensor_mul(xln[:, ct, tsl], t[:], RSTD[:])
                return xln

            def mix(xln, maa_t, kind, tch):
                """m = xln*(1-maa) + shift(xln)*maa for tokens [tch*512, +512)"""
                m = mixp.tile([128, CT, 512], bf16, tag="mix")
                lo = tch * 512
                for ct in range(CT):
                    nc.vector.tensor_scalar_mul(
                        m[:, ct, :], xln[:, ct, lo:lo + 512],
                        maa_t[:, ct, 6 + kind:7 + kind])
                    if tch == 0:
                        nc.vector.scalar_tensor_tensor(
                            m[:, ct, 1:512], xln[:, ct, 0:511],
                            maa_t[:, ct, kind:kind + 1], m[:, ct, 1:512],
                            OP.mult, OP.add)
                    else:
                        nc.vector.scalar_tensor_tensor(
                            m[:, ct, :], xln[:, ct, lo - 1:lo + 511],
                            maa_t[:, ct, kind:kind + 1], m[:, ct, :],
                            OP.mult, OP.add)
                return m

            def load_w(name, l, fsl=None):
                t = wp.tile([128, CT, C], bf16, tag="wcc")
                if fsl is None:
                    nc.sync.dma_start(t[:], G[name][l])
                else:
                    nc.sync.dma_start(t[:], G[name][l, :, :, fsl])
                return t

            # ============== layers ==============
            for l in range(n_layers):
                maa_t = lcon.tile([128, CT, 12], f32, tag="maa")
                nc.sync.dma_start(maa_t[:], VF["maa"][l])
                bias_t = lcon.tile([128, CT, 8], f32, tag="bias")
                nc.sync.dma_start(bias_t[:], VF["bias_cm"][l])
                bckt = lcon.tile([128, HT], f32, tag="bck")
                nc.sync.dma_start(bckt[:], VF["bck_t"][l])
                lnwh_t = lcon.tile([128, H], f32, tag="lnwh")
                nc.sync.dma_start(lnwh_t[:], VF["lnw_h"][l])
                uh_t = lcon.tile([128, H], f32, tag="uh")
                nc.sync.dma_start(uh_t[:], VF["u_h"][l])
                lnwct_t = lcon.tile([128, CT], f32, tag="lnwct")
                nc.sync.dma_start(lnwct_t[:], VF["lnw_ct"][l])
                # decay tables: wmT[p,h,jt,i] = exp(min(lnw_h*(i-j-1), 0))
                #   * [i>j] + u_h*[i==j]  (j = jt*128+p);  wbq = exp(lnw*i)
                wmT = lcon.tile([128, H, 2, Q], bf16, tag="wmT")
                for h in range(H):
                    for jt in range(2):
                        es = tmpp.tile([128, Q], f32, tag="wmes", bufs=1)
                        nc.vector.tensor_scalar(
                            es[:], emat_t[:, jt, :], lnwh_t[:, h:h + 1], 0.0,
                            OP.mult, OP.min)
                        pe = tmpp.tile([128, Q], bf16, tag="wmpe", bufs=1)
                        nc.scalar.activation(pe[:], es[:], AF.Exp)
                        pm = tmpp.tile([128, Q], bf16, tag="wmpm", bufs=1)
                        nc.vector.tensor_mul(pm[:], pe[:], mtri_t[:, jt, :])
                        nc.vector.scalar_tensor_tensor(
                            wmT[:, h, jt, :], meye_t[:, jt, :],
                            uh_t[:, h:h + 1], pm[:], OP.mult, OP.add)
                wbq = lcon.tile([128, CT, Q], bf16, tag="wbq")
                for ct in range(CT):
                    es = tmpp.tile([128, Q], f32, tag="wmes", bufs=1)
                    nc.vector.tensor_scalar(
                        es[:], irow_t[:], lnwct_t[:, ct:ct + 1], 0.0,
                        OP.mult, OP.min)
                    nc.scalar.activation(wbq[:, ct, :], es[:], AF.Exp)
                wkc = lcon.tile([128, H * 2], f32, tag="wk")
                nc.sync.dma_start(wkc[:], VF["wk_col"][l])
                wsbc = lcon.tile([128, H], f32, tag="ws")
                nc.sync.dma_start(wsbc[:], VF["ws_bc"][l])
                brow0 = lcon.tile([1, C], bf16, tag="brow0")
                nc.sync.dma_start(brow0[:], VH["bias_rows"][l, 0:1, :])
                brow1 = lcon.tile([1, C], bf16, tag="brow1")
                nc.sync.dma_start(brow1[:], VH["bias_rows"][l, 1:2, :])

                # ---- tmix ----
                xln = layernorm(x_res)
                kC = actp.tile([128, CT, T], bf16, tag="kh", bufs=1)
                kT = actp.tile([128, TT, C], bf16, tag="kT", bufs=1)
                vT = actp.tile([128, TT, C], bf16, tag="vT", bufs=1)
                r_sb = actp.tile([128, CT, T], bf16, tag="r", bufs=1)
                gC = actp.tile([128, CT, T], bf16, tag="gx", bufs=1)

                def proj_cm_dst(mx, wt, dst, func, bkind):
                    for mt in range(CT):
                        for tch in range(2):
                            tsl = slice(tch * 512, (tch + 1) * 512)
                            ps = ps512.tile([128, 512], f32, tag="ps512")
                            for kt in range(CT):
                                nc.tensor.matmul(
                                    ps[:], wt[:, kt, mt * 128:(mt + 1) * 128],
                                    mx[tch][:, kt, :],
                                    start=(kt == 0), stop=(kt == CT - 1))
                            if func == "silu":
                                sg = tmpp.tile([128, 512], bf16, tag="sg", bufs=1)
                                nc.scalar.activation(sg[:], ps[:], AF.Sigmoid,
                                                     bias=bias_t[:, mt, bkind:bkind + 1])
                                nc.vector.scalar_tensor_tensor(
                                    dst[:, mt, tsl], ps[:],
                                    bias_t[:, mt, bkind:bkind + 1], sg[:],
                                    OP.add, OP.mult)
                            else:
                                nc.scalar.activation(dst[:, mt, tsl], ps[:], func,
                                                     bias=bias_t[:, mt, bkind:bkind + 1])

                def proj_tm_dst(mx, wt, dst, brow):
                    for tch in range(2):
                        for tt4 in range(4):
                            tt = tch * 4 + tt4
                            for nch in range(2):
                                nsl = slice(nch * 384, (nch + 1) * 384)
                                ps = ps512.tile([128, 512], f32, tag="ps512")
                                for kt in range(CT):
                                    nc.tensor.matmul(
                                        ps[:, 0:384],
                                        mx[tch][:, kt, tt4 * 128:(tt4 + 1) * 128],
                                        wt[:, kt, nsl], start=(kt == 0), stop=False)
                                nc.tensor.matmul(ps[:, 0:384], ones_row_h[:],
                                                 brow[0:1, nsl], start=False,
                                                 stop=True)
                                nc.scalar.activation(dst[:, tt, nsl], ps[:, 0:384],
                                                     AF.Copy)

                wk_t = load_w("Wk", l)
                xk = [mix(xln, maa_t, 0, 0), mix(xln, maa_t, 0, 1)]
                proj_cm_dst(xk, wk_t, kC, AF.Identity, 1)
                proj_tm_dst(xk, wk_t, kT, brow0)
                wv_t = load_w("Wv", l)
                xv = [mix(xln, maa_t, 1, 0), mix(xln, maa_t, 1, 1)]
                proj_tm_dst(xv, wv_t, vT, brow1)
                wr_t = load_w("Wr", l)
                xr = [mix(xln, maa_t, 2, 0), mix(xln, maa_t, 2, 1)]
                proj_cm_dst(xr, wr_t, r_sb, AF.Identity, 0)
                wg_t = load_w("Wg", l)
                xg = [mix(xln, maa_t, 3, 0), mix(xln, maa_t, 3, 1)]
                proj_cm_dst(xg, wg_t, gC, "silu", 6)

                # ---- chunked attention ----
                wo_t = load_w("Wo", l)
                for ch in range(NCH):
                    csl = slice(ch * Q, (ch + 1) * Q)
                    yT = chp.tile([128, CT, Q], bf16, tag="yT")
                    for ct in range(CT):
                        if ch > 0:
                            rwf = tmpp.tile([128, Q], bf16, tag="rwf", bufs=2)
                            nc.vector.tensor_mul(rwf[:], r_sb[:, ct, csl],
                                                 wbq[:, ct, :])
                        y_ps = ps512.tile([128, 512], f32, tag="ps512")
                        for hh in range(2):
                            h = 2 * ct + hh
                            hp = hh * 64
                            a_ps = ps512.tile([128, 512], f32, tag="ps512")
                            for jt in range(2):
                                nc.tensor.matmul(
                                    a_ps[:, jt * Q:(jt + 1) * Q],
                                    kC[hp:hp + 64, ct,
                                       (2 * ch + jt) * 128:(2 * ch + jt + 1) * 128],
                                    r_sb[hp:hp + 64, ct, csl],
                                    start=True, stop=True)
                            a_sb = tmpp.tile([128, 512], bf16, tag="attT", bufs=2)
                            for jt in range(2):
                                nc.vector.tensor_mul(
                                    a_sb[:, jt * Q:(jt + 1) * Q],
                                    a_ps[:, jt * Q:(jt + 1) * Q], wmT[:, h, jt, :])
                            ysl = y_ps[hp:hp + 64, 0:Q]
                            for jt in range(2):
                                nc.tensor.matmul(
                                    ysl, vT[:, 2 * ch + jt, h * 64:(h + 1) * 64],
                                    a_sb[:, jt * Q:(jt + 1) * Q],
                                    start=(jt == 0),
                                    stop=(jt == 1 and ch == 0))
                            if ch > 0:
                                nc.tensor.matmul(ysl, S_b[hp:hp + 64, h, :],
                                                 rwf[hp:hp + 64, :],
                                                 start=False, stop=True)
                            kw = tmpp.tile([128, 2, HD], bf16, tag="kw", bufs=1)
                            for jt in range(2):
                                nc.vector.tensor_scalar_mul(
                                    kw[:, jt, :],
                                    kT[:, 2 * ch + jt, h * 64:(h + 1) * 64],
                                    wkc[:, h * 2 + jt:h * 2 + jt + 1])
                            c_ps = pss.tile([128, 512], f32, tag="pss")
                            for jt in range(2):
                                nc.tensor.matmul(
                                    c_ps[hp:hp + 64, 0:64], kw[:, jt, :],
                                    vT[:, 2 * ch + jt, h * 64:(h + 1) * 64],
                                    start=(jt == 0), stop=(jt == 1))
                            if ch == 0:
                                nc.vector.tensor_copy(S_f[hp:hp + 64, h, :],
                                                      c_ps[hp:hp + 64, 0:64])
                            else:
                                nc.vector.scalar_tensor_tensor(
                                    S_f[hp:hp + 64, h, :], S_f[hp:hp + 64, h, :],
                                    wsbc[hp:hp + 64, h:h + 1],
                                    c_ps[hp:hp + 64, 0:64], OP.mult, OP.add)
                            if ch < NCH - 1:
                                nc.vector.tensor_copy(S_b[hp:hp + 64, h, :],
                                                      S_f[hp:hp + 64, h, :])
                        nc.scalar.activation(yT[:, ct, :], y_ps[:, 0:Q], AF.Copy)

                    # GroupNorm: two-pass (center in place, then sum of squares)
                    mu_ps = pss.tile([128, 512], f32, tag="pss")
                    for ct in range(CT):
                        nc.tensor.matmul(mu_ps[0:H, 0:Q], blka[:, ct, :], yT[:, ct, :],
                                         start=(ct == 0), stop=(ct == CT - 1))
                    mu_sb = sm.tile([12, Q], f32, tag="gvar", bufs=3)
                    nc.scalar.activation(mu_sb[:], mu_ps[0:H, 0:Q], AF.Copy,
                                         scale=1.0 / HD)
                    for ct in range(CT):
                        MUb = ps512.tile([128, 512], f32, tag="ps512")
                        nc.tensor.matmul(MUb[:, 0:Q], blkb[:, ct, :], mu_sb[:],
                                         start=True, stop=True)
                        nc.vector.tensor_sub(yT[:, ct, :], yT[:, ct, :], MUb[:, 0:Q])
                    m2_ps = pss.tile([128, 512], f32, tag="pss")
                    for ct in range(CT):
                        sq = tmpp.tile([128, Q], f32, tag="gnsq", bufs=2)
                        nc.vector.tensor_mul(sq[:], yT[:, ct, :], yT[:, ct, :])
                        nc.tensor.matmul(m2_ps[0:H, 0:Q], blkaf[:, ct, :], sq[:],
                                         start=(ct == 0), stop=(ct == CT - 1))
                    var_sb = sm.tile([12, Q], f32, tag="gvar", bufs=3)
                    nc.scalar.activation(var_sb[:], m2_ps[0:H, 0:Q], AF.Copy,
                                         scale=1.0 / HD)
                    std_sb = sm.tile([12, Q], f32, tag="gvar", bufs=3)
                    nc.scalar.activation(std_sb[:], var_sb[:], AF.Sqrt,
                                         bias=eps_gn[0:12, :])
                    rstd_sb = sm.tile([12, Q], f32, tag="gvar", bufs=3)
                    nc.vector.reciprocal(rstd_sb[:], std_sb[:])
                    prod = chp.tile([128, CT, Q], bf16, tag="prod")
                    for ct in range(CT):
                        RSb = ps512.tile([128, 512], f32, tag="ps512")
                        nc.tensor.matmul(RSb[:, 0:Q], blkb[:, ct, :], rstd_sb[:],
                                         start=True, stop=True)
                        t2 = tmpp.tile([128, Q], f32, tag="gnt", bufs=2)
                        nc.vector.tensor_mul(t2[:], yT[:, ct, :], RSb[:, 0:Q])
                        nc.vector.scalar_tensor_tensor(
                            prod[:, ct, :], t2[:], bias_t[:, ct, 5:6],
                            gC[:, ct, csl], OP.add, OP.mult)
                    for mt in range(CT):
                        ps = ps512.tile([128, 512], f32, tag="ps512")
                        for kt in range(CT):
                            nc.tensor.matmul(
                                ps[:, 0:Q], wo_t[:, kt, mt * 128:(mt + 1) * 128],
                                prod[:, kt, :], start=(kt == 0), stop=(kt == CT - 1))
                        nc.vector.scalar_tensor_tensor(
                            x_res[:, mt, csl], ps[:, 0:Q], bias_t[:, mt, 2:3],
                            x_res[:, mt, csl], OP.add, OP.add)

                # ---- cmix ----
                xln2 = layernorm(x_res)
                wcr_t = load_w("Wcr", l)
                xr2 = [mix(xln2, maa_t, 5, 0), mix(xln2, maa_t, 5, 1)]
                gate = actp.tile([128, CT, T], bf16, tag="gx", bufs=1)
                for mt in range(CT):
                    for tch in range(2):
                        tsl = slice(tch * 512, (tch + 1) * 512)
                        ps = ps512.tile([128, 512], f32, tag="ps512")
                        for kt in range(CT):
                            nc.tensor.matmul(
                                ps[:], wcr_t[:, kt, mt * 128:(mt + 1) * 128],
                                xr2[tch][:, kt, :],
                                start=(kt == 0), stop=(kt == CT - 1))
                        nc.scalar.activation(gate[:, mt, tsl], ps[:], AF.Sigmoid,
                                             bias=bias_t[:, mt, 4:5])
                xk2 = [mix(xln2, maa_t, 4, 0), mix(xln2, maa_t, 4, 1)]
                for tch in range(2):
                    tsl = slice(tch * 512, (tch + 1) * 512)
                    h2 = actp.tile([128, HT, 512], bf16, tag="kh", bufs=1)
                    for third in range(3):
                        wck_t = load_w("Wck", l, slice(third * C, (third + 1) * C))
                        for mt6 in range(6):
                            gmt = third * 6 + mt6
                            ps = ps512.tile([128, 512], f32, tag="ps512")
                            for kt in range(CT):
                                nc.tensor.matmul(
                                    ps[:], wck_t[:, kt, mt6 * 128:(mt6 + 1) * 128],
                                    xk2[tch][:, kt, :],
                                    start=(kt == 0), stop=(kt == CT - 1))
                            hr = tmpp.tile([128, 512], bf16, tag="hrelu", bufs=2)
                            nc.vector.tensor_scalar(
                                hr[:], ps[:], bckt[:, gmt:gmt + 1], 0.0,
                                OP.add, OP.max)
                            nc.vector.tensor_mul(h2[:, gmt, :], hr[:], hr[:])
                    for third in range(3):
                        wcv_t = wp.tile([128, HT, Q], bf16, tag="wcc")
                        nc.sync.dma_start(
                            wcv_t[:],
                            G["Wcv"][l, :, :, third * Q:(third + 1) * Q])
                        for mt2 in range(2):
                            gmt = third * 2 + mt2
                            ps = ps512.tile([128, 512], f32, tag="ps512")
                            for kt in range(HT):
                                nc.tensor.matmul(
                                    ps[:], wcv_t[:, kt, mt2 * 128:(mt2 + 1) * 128],
                                    h2[:, kt, :], start=(kt == 0), stop=(kt == HT - 1))
                            t = tmpp.tile([128, 512], f32, tag="cvt", bufs=1)
                            nc.vector.scalar_tensor_tensor(
                                t[:], ps[:], bias_t[:, gmt, 3:4], gate[:, gmt, tsl],
                                OP.add, OP.mult)
                            nc.vector.tensor_add(x_res[:, gmt, tsl],
                                                 x_res[:, gmt, tsl], t[:])

            # ============== head ==============
            sq_sb = sm.tile([128, CT, 2], f32, tag="hsq")
            for ct in range(CT):
                nc.vector.tensor_mul(sq_sb[:, ct, 1:2], x_res[:, ct, T - 1:T],
                                     x_res[:, ct, T - 1:T])
                nc.vector.tensor_copy(sq_sb[:, ct, 0:1], x_res[:, ct, T - 1:T])
            mu_ps = pss.tile([128, 512], f32, tag="pss")
            for ct in range(CT):
                nc.tensor.matmul(mu_ps[0:1, 0:2], ones_col[:], sq_sb[:, ct, :],
                                 start=(ct == 0), stop=(ct == CT - 1))
            st_row = sm.tile([1, 2], f32, tag="hrow", bufs=4)
            nc.scalar.activation(st_row[:], mu_ps[0:1, 0:2], AF.Copy, scale=1.0 / C)
            mu2_row = sm.tile([1, 1], f32, tag="hrow", bufs=4)
            nc.vector.tensor_mul(mu2_row[:], st_row[:, 0:1], st_row[:, 0:1])
            var_row = sm.tile([1, 1], f32, tag="hrow", bufs=4)
            nc.vector.tensor_sub(var_row[:], st_row[:, 1:2], mu2_row[:])
            stdh_row = sm.tile([1, 1], f32, tag="hrow", bufs=4)
            nc.scalar.activation(stdh_row[:], var_row[:], AF.Sqrt,
                                 bias=eps_ln[0:1, :])
            rstd_row = sm.tile([1, 1], f32, tag="hrow", bufs=4)
            nc.vector.reciprocal(rstd_row[:], stdh_row[:])
            MU128 = sm.tile([128, 1], f32, tag="hb")
            RSTD128 = sm.tile([128, 1], f32, tag="hb")
            nc.gpsimd.partition_broadcast(MU128[:], st_row[:, 0:1])
            nc.gpsimd.partition_broadcast(RSTD128[:], rstd_row[:])
            xl = sm.tile([128, CT], bf16, tag="xl")
            for ct in range(CT):
                nc.vector.scalar_tensor_tensor(
                    xl[:, ct:ct + 1], x_res[:, ct, T - 1:T], MU128[:],
                    RSTD128[:], OP.subtract, OP.mult)
            nv = (V + 511) // 512
            for nt in range(nv):
                nsz = min(512, V - nt * 512)
                ps = pss.tile([128, 512], f32, tag="pss")
                for kt in range(CT):
                    wv_sb = tmpp.tile([128, 512], bf16, tag="hw", bufs=2)
                    nc.sync.dma_start(
                        wv_sb[:, 0:nsz],
                        G["wteT"][:, kt, nt * 512:nt * 512 + nsz])
                    nc.tensor.matmul(ps[0:1, 0:nsz], xl[:, kt:kt + 1],
                                     wv_sb[:, 0:nsz],
                                     start=(kt == 0), stop=(kt == CT - 1))
                ot = sm.tile([1, 512], f32, tag="hout")
                nc.scalar.activation(ot[:, 0:nsz], ps[0:1, 0:nsz], AF.Copy)
                nc.sync.dma_start(out_logits.ap()[:, nt * 512:nt * 512 + nsz],
                                  ot[:, 0:nsz])

    nc.compile()
    return nc


def _get_program(n_layers=L):
    if n_layers not in _PROG_CACHE:
        _PROG_CACHE[n_layers] = _build_program(n_layers)
    return _PROG_CACHE[n_layers]


def kernel(**inputs):
    from concourse.bass_utils import run_bass_kernel_spmd

    in_maps, lbias = _host_precompute(inputs)
    nc = _get_program(L)
    res = run_bass_kernel_spmd(nc, in_maps, core_ids=list(range(NCORES)),
                               trace=False)
    out = np.zeros((B, 1, V), np.float32)
    for b in range(B):
        out[b, 0, :] = res.results[b]["logits"][0]
    out += lbias[None, None, :]
    return out


# revision 33
# speedup vs baseline: 1.0869x; 1.0100x over previous
# nn_GPT_64347200029289 — RWKV6-style dense transformer on 8 TRN2 NeuronCores.
# B=4, T=1024, C=768, H=12 heads (headdim 64), L=12 layers, V=50304.
# Output: last-position logits [B, 1, V].
#
# Sharding: the host→device tunnel is the bottleneck (~75 MB/s effective), so
# all large weight tensors are shipped 8-way sharded (each core receives a
# distinct 1/8 slice) and reassembled on-device with AllGather collectives
# over NeuronLink. After the gather every core holds the full weights in
# internal DRAM; core c then runs the full 12-layer body for batch c%4 and
# computes full-vocab logits (cores 4-7 duplicate 0-3; host keeps cores 0-3).
#
# Layout: residual kept C-major ([C-tile=128 partitions, T free], fp32).
# All matmuls bf16 with fp32 PSUM accumulation. LayerNorm/GroupNorm affine
# params are folded into adjacent projection weights on the host. LN stats are
# partition reductions via ones-matmuls; GroupNorm stats use block-diagonal
# ones-matmuls so y stays C-major (no transposes anywhere). Attention is the
# chunked RWKV scan (Q=256, 4 chunks) with host-precomputed decay tables.

import sys
import numpy as np

sys.path.insert(0, "/opt/trn_rl_repo")

import ml_dtypes

# Persistent jax compilation cache: run_bass_kernel_spmd builds a fresh
# jax.jit per call, which otherwise re-runs the ~5s neuronxcc compile of the
# (unchanged) NEFF on every call.
try:
    import jax
    jax.config.update("jax_compilation_cache_dir", "/tmp/jaxcache")
    jax.config.update("jax_persistent_cache_min_compile_time_secs", 0.0)
    jax.config.update("jax_persistent_cache_min_entry_size_bytes", 0)
except Exception:
    pass

C, H, L, V, BLK = 768, 12, 12, 50304, 1024
HD = C // H                  # 64
B, T, Q = 4, 1024, 256
NCH = T // Q                 # 4 chunks
CT = C // 128                # 6
TT = T // 128                # 8
HT = (3 * C) // 128          # 18
GN_EPS = 1e-5 * 64
LN_EPS = 1e-5
NCORES = 8

BF = np.float16

# name -> (natural shape, 2D collective shape [rows, cols]); rows % 8 == 0,
# rank-r shard is rows/8 consecutive rows. Gathered tensor layout == natural.
SHARD_SPECS = {
    "Wr":   ((L, 128, CT, C), (L * 128 * CT, C)),
    "Wk":   ((L, 128, CT, C), (L * 128 * CT, C)),
    "Wv":   ((L, 128, CT, C), (L * 128 * CT, C)),
    "Wg":   ((L, 128, CT, C), (L * 128 * CT, C)),
    "Wo":   ((L, 128, CT, C), (L * 128 * CT, C)),
    "Wcr":  ((L, 128, CT, C), (L * 128 * CT, C)),
    "Wck":  ((L, 128, CT, 3 * C), (L * 128 * CT, 3 * C)),
    "Wcv":  ((L, 128, HT, C), (L * 128 * HT, C)),
    "wteT": ((128, CT, V), (128 * CT, V)),
}

# Small replicated tables, packed into one f32 + one fp16 blob that are
# 8-way sharded and AllGathered like the big weights (order matters: the
# program reconstructs views at these offsets).
F32_SMALLS = [
    ("maa", (L, 128, CT, 12)), ("bias_cm", (L, 128, CT, 8)),
    ("bck_t", (L, 128, HT)), ("wk_col", (L, 128, H * 2)),
    ("ws_bc", (L, 128, H)), ("blk_af", (128, CT, H)),
    ("blk_b", (12, CT, 128)), ("lnw_h", (L, 128, H)),
    ("u_h", (L, 128, H)), ("lnw_ct", (L, 128, CT)),
]
H16_SMALLS = [
    ("bias_rows", (L, 2, C)), ("blk_a", (128, CT, H)),
    ("emat", (128, 2, Q)), ("mtri", (128, 2, Q)), ("meye", (128, 2, Q)),
    ("irow", (128, Q)),
]


def _blob_pack(smalls, specs, dtype):
    parts = [np.ascontiguousarray(smalls[n], dtype).ravel() for n, s in specs]
    flat = np.concatenate(parts)
    pad = (-flat.size) % (NCORES * 128)
    if pad:
        flat = np.concatenate([flat, np.zeros(pad, dtype)])
    return flat.reshape(-1, 128)


def _host_precompute(inputs):
    f = lambda k: np.asarray(inputs[k], np.float32)
    idx = np.asarray(inputs["idx"])
    wte, wpe = f("wte"), f("wpe")
    ln1_w, ln1_b = f("ln1_w"), f("ln1_b")
    ln2_w, ln2_b = f("ln2_w"), f("ln2_b")
    gn_w, gn_b = f("gn_w"), f("gn_b")
    lnf_w, lnf_b = f("lnf_w"), f("lnf_b")
    Wr, Wk, Wv, Wg, Wo = f("Wr"), f("Wk"), f("Wv"), f("Wg"), f("Wo")
    Wck, Wcv, Wcr = f("Wck"), f("Wcv"), f("Wcr")
    br, bk, bv, bg, bo = f("br"), f("bk"), f("bv"), f("bg"), f("bo")
    bck, bcv, bcr = f("bck"), f("bcv"), f("bcr")
    maa_tk, maa_tv = f("maa_tk"), f("maa_tv")
    maa_tr, maa_tg = f("maa_tr"), f("maa_tg")
    cmaa_k, cmaa_r = f("cmaa_k"), f("cmaa_r")
    tdecay, tfaaaa = f("tdecay"), f("tfaaaa")

    def fold(W, lw, lb, bproj):
        We = lw[:, :, None] * W
        be = bproj + np.einsum("lc,lco->lo", lb, W)
        return We, be

    Wr_e, br_e = fold(Wr, ln1_w, ln1_b, br)
    Wk_e, bk_e = fold(Wk, ln1_w, ln1_b, bk)
    Wv_e, bv_e = fold(Wv, ln1_w, ln1_b, bv)
    Wg_e, bg_e = fold(Wg, ln1_w, ln1_b, bg)
    Wck_e, bck_e = fold(Wck, ln2_w, ln2_b, bck)
    Wcr_e, bcr_e = fold(Wcr, ln2_w, ln2_b, bcr)
    Wo_e = gn_w[:, :, None] * Wo
    Wcv_e = Wcv
    bo_e, bcv_e = bo, bcv

    w = np.exp(-np.exp(tdecay)).astype(np.float64)       # [L,H]
    ii = np.arange(Q)
    wk_ = (w[:, :, None] ** (Q - 1 - ii)[None, None, :]).astype(np.float32)
    ws_ = (w ** Q).astype(np.float32)

    wteT_e = lnf_w[:, None] * wte.T                      # [C,V]
    lbias = lnf_b @ wte.T                                # [V]
    x0 = wte[idx] + wpe[:T]                              # [B,T,C]

    def cm(M):  # [Cin,F] -> [128, Cin//128, F]
        Cin, F2 = M.shape
        return np.ascontiguousarray(M.reshape(Cin // 128, 128, F2).transpose(1, 0, 2))

    big = {}
    for name, We in (("Wr", Wr_e), ("Wk", Wk_e), ("Wv", Wv_e), ("Wg", Wg_e),
                     ("Wo", Wo_e), ("Wcr", Wcr_e), ("Wck", Wck_e), ("Wcv", Wcv_e)):
        big[name] = np.stack([cm(We[l]) for l in range(L)]).astype(BF)
    big["wteT"] = cm(wteT_e).astype(BF)                  # [128, CT, V]

    # wmT/wbq decay tables are generated on-device from lnw = ln(w) =
    # -exp(tdecay): wmT[l,p,h,jt,i] = exp(min(lnw*(i-j-1),0))*[i>j] + u*[i==j]
    # with j = jt*128+p; wbq[l,p,ct,i] = exp(lnw_head(ct,p) * i).
    lnw = (-np.exp(tdecay)).astype(np.float32)           # [L,H]
    lnw_h = np.ascontiguousarray(
        np.broadcast_to(lnw[:, None, :], (L, 128, H))).astype(np.float32)
    u_h = np.ascontiguousarray(
        np.broadcast_to(tfaaaa[:, None, :], (L, 128, H))).astype(np.float32)
    lnw_ct = np.zeros((L, 128, CT), np.float32)
    for ct in range(CT):
        lnw_ct[:, 0:64, ct] = lnw[:, 2 * ct, None]
        lnw_ct[:, 64:128, ct] = lnw[:, 2 * ct + 1, None]
    ivec = np.arange(Q, dtype=np.float32)
    jvec = np.arange(128, dtype=np.float32)
    emat = np.zeros((128, 2, Q), np.float32)
    mtri = np.zeros((128, 2, Q), np.float32)
    meye = np.zeros((128, 2, Q), np.float32)
    for jt in range(2):
        jj = jt * 128 + jvec[:, None]
        emat[:, jt, :] = ivec[None, :] - jj - 1.0
        mtri[:, jt, :] = (ivec[None, :] > jj).astype(np.float32)
        meye[:, jt, :] = (ivec[None, :] == jj).astype(np.float32)
    irow = np.broadcast_to(ivec[None, :], (128, Q))

    wk_col = wk_.reshape(L, H, 2, 128).transpose(0, 3, 1, 2).reshape(L, 128, H * 2)
    ws_bc = np.zeros((L, 128, H), np.float32)
    ws_bc[:, 0:64, :] = ws_[:, None, :]
    ws_bc[:, 64:128, :] = ws_[:, None, :]

    # mix coefficients [L,128,CT,12]: kinds tk,tv,tr,tg,ck,cr then negated
    maa_all = np.stack([maa_tk, maa_tv, maa_tr,
                        maa_tg, cmaa_k, cmaa_r], axis=-1)   # [L,C,6]
    maa_all = np.concatenate([maa_all, 1.0 - maa_all], axis=-1)
    maa_pack = maa_all.reshape(L, CT, 128, 12).transpose(0, 2, 1, 3)

    # C-major per-partition biases [L,128,CT,8]: br,bkC,bo,bcv,bcr,gnb,bg,pad
    bias_cm = np.stack([br_e, bk_e, bo_e, bcv_e, bcr_e,
                        np.broadcast_to(gn_b, br_e.shape), bg_e,
                        np.zeros_like(br_e)], axis=-1)
    bias_cm = bias_cm.reshape(L, CT, 128, 8).transpose(0, 2, 1, 3)
    bck_t = bck_e.reshape(L, HT, 128).transpose(0, 2, 1)          # [L,128,HT]
    bias_rows = np.stack([bk_e, bv_e], axis=1)                    # [L,2,C]

    blk_a = np.zeros((128, CT, H), np.float32)
    blk_b = np.zeros((12, CT, 128), np.float32)
    for ct in range(CT):
        blk_a[0:64, ct, 2 * ct] = 1.0
        blk_a[64:128, ct, 2 * ct + 1] = 1.0
        blk_b[2 * ct, ct, 0:64] = 1.0
        blk_b[2 * ct + 1, ct, 64:128] = 1.0

    smalls = {
        "maa": maa_pack, "bias_cm": bias_cm, "bck_t": bck_t,
        "bias_rows": bias_rows, "wk_col": wk_col, "ws_bc": ws_bc,
        "blk_a": blk_a, "blk_af": blk_a, "blk_b": blk_b,
        "lnw_h": lnw_h, "u_h": u_h, "lnw_ct": lnw_ct,
        "emat": emat, "mtri": mtri, "meye": meye, "irow": irow,
    }
    fblob = _blob_pack(smalls, F32_SMALLS, np.float32)
    hblob = _blob_pack(smalls, H16_SMALLS, BF)
    frs, hrs = fblob.shape[0] // NCORES, hblob.shape[0] // NCORES
    common = {}

    # 1/8 row-shards of each big tensor (concat over ranks == natural layout)
    for name, (nat, two_d) in SHARD_SPECS.items():
        rows, cols = two_d
        arr = big[name].reshape(rows, cols)
        rs = rows // NCORES
        big[name] = [np.ascontiguousarray(arr[c * rs:(c + 1) * rs])
                     for c in range(NCORES)]

    x0cm = [np.ascontiguousarray(
        x0[b].T.reshape(CT, 128, T).transpose(1, 0, 2)).astype(BF)
        for b in range(4)]
    # Cores 4-7 duplicate batches 0-3 and their logits are discarded; zeros
    # stage much faster through the transport and stay finite through the
    # eps-guarded norms.
    x0_zero = np.zeros_like(x0cm[0])
    in_maps = []
    for c in range(NCORES):
        b = c % 4
        m = dict(common)
        for name in SHARD_SPECS:
            m[name + "_sh"] = big[name][c]
        m["fsm_sh"] = np.ascontiguousarray(fblob[c * frs:(c + 1) * frs])
        m["hsm_sh"] = np.ascontiguousarray(hblob[c * hrs:(c + 1) * hrs])
        m["x0"] = x0cm[b] if c < 4 else x0_zero
        in_maps.append(m)
    return in_maps, lbias


# ---------------------------------------------------------------------------

_PROG_CACHE = {}


def _build_program(n_layers=L):
    import concourse.bass as bass
    import concourse.tile as tile
    from concourse import mybir, bacc
    from contextlib import ExitStack

    f32 = mybir.dt.float32
    bf16 = mybir.dt.float16
    AF = mybir.ActivationFunctionType
    OP = mybir.AluOpType

    nc = bacc.Bacc("TRN2", target_bir_lowering=False, debug=False,
                   num_devices=NCORES)

    dram = {}
    def din(name, shape, dt=bf16):
        dram[name] = nc.dram_tensor(name, list(shape), dt, kind="ExternalInput")

    din("x0", (128, CT, T))
    for name, (nat, (rows, cols)) in SHARD_SPECS.items():
        din(name + "_sh", (rows // NCORES, cols))
    nf32 = sum(int(np.prod(s)) for _, s in F32_SMALLS)
    nh16 = sum(int(np.prod(s)) for _, s in H16_SMALLS)
    frows = -(-nf32 // 128 // NCORES) * NCORES
    hrows = -(-nh16 // 128 // NCORES) * NCORES
    din("fsm_sh", (frows // NCORES, 128), f32)
    din("hsm_sh", (hrows // NCORES, 128))
    out_logits = nc.dram_tensor("logits", [1, V], f32, kind="ExternalOutput")

    with tile.TileContext(nc) as tc:
        with ExitStack() as ctx:
            dpool = ctx.enter_context(tc.tile_pool(name="dpool", bufs=1,
                                                   space="DRAM"))
            pers = ctx.enter_context(tc.tile_pool(name="pers", bufs=1))
            lcon = ctx.enter_context(tc.tile_pool(name="lcon", bufs=1))
            wp = ctx.enter_context(tc.tile_pool(name="wp", bufs=2))
            actp = ctx.enter_context(tc.tile_pool(name="actp", bufs=1))
            mixp = ctx.enter_context(tc.tile_pool(name="mixp", bufs=3))
            chp = ctx.enter_context(tc.tile_pool(name="chp", bufs=1))
            sm = ctx.enter_context(tc.tile_pool(name="sm", bufs=2))
            tmpp = ctx.enter_context(tc.tile_pool(name="tmpp", bufs=2))
            ps512 = ctx.enter_context(tc.tile_pool(name="ps512", bufs=5, space="PSUM"))
            pss = ctx.enter_context(tc.tile_pool(name="pss", bufs=3, space="PSUM"))

            # -------- gather the sharded weights over NeuronLink --------
            G = {}
            rg = [list(range(NCORES))]
            merge = {
                "Wr": "l p ct c -> (l p ct) c", "Wk": "l p ct c -> (l p ct) c",
                "Wv": "l p ct c -> (l p ct) c", "Wg": "l p ct c -> (l p ct) c",
                "Wo": "l p ct c -> (l p ct) c", "Wcr": "l p ct c -> (l p ct) c",
                "Wck": "l p ct c -> (l p ct) c", "Wcv": "l p ct c -> (l p ct) c",
                "wteT": "p ct v -> (p ct) v",
            }
            for name, (nat, (rows, cols)) in SHARD_SPECS.items():
                bounce = dpool.tile([rows // NCORES, cols], bf16)
                nc.gpsimd.dma_start(bounce[:], dram[name + "_sh"].ap())
                full = dpool.tile(list(nat), bf16, addr_space="Shared")
                nc.gpsimd.collective_compute(
                    "AllGather", mybir.AluOpType.bypass,
                    replica_groups=rg,
                    ins=[bounce[:]],
                    outs=[full.rearrange(merge[name])],
                )
                G[name] = full

            def gather_blob(inp_name, n_rows, dt, specs):
                b_ = dpool.tile([n_rows // NCORES, 128], dt)
                nc.gpsimd.dma_start(b_[:], dram[inp_name].ap())
                fl = dpool.tile([n_rows, 128], dt, addr_space="Shared")
                nc.gpsimd.collective_compute(
                    "AllGather", mybir.AluOpType.bypass, replica_groups=rg,
                    ins=[b_[:]], outs=[fl[:]])
                flat = fl.rearrange("r c -> (r c)")
                views, off = {}, 0
                for nm, shape in specs:
                    n = int(np.prod(shape))
                    pat = ("(" + " ".join(f"d{i}" for i in range(len(shape)))
                           + ") -> " + " ".join(f"d{i}" for i in range(len(shape))))
                    views[nm] = flat[off:off + n].rearrange(
                        pat, **{f"d{i}": s for i, s in enumerate(shape)})
                    off += n
                return views
            VF = gather_blob("fsm_sh", frows, f32, F32_SMALLS)
            VH = gather_blob("hsm_sh", hrows, bf16, H16_SMALLS)

            x_res = pers.tile([128, CT, T], f32)
            x0_sb = actp.tile([128, CT, T], bf16, tag="xln", bufs=1)
            nc.sync.dma_start(x0_sb[:], dram["x0"].ap())
            for ct in range(CT):
                nc.vector.tensor_copy(x_res[:, ct, :], x0_sb[:, ct, :])
            S_f = pers.tile([128, H, HD], f32)
            S_b = pers.tile([128, H, HD], bf16)
            ones_col = pers.tile([128, 1], f32)
            nc.gpsimd.memset(ones_col[:], 1.0)
            ones_row = pers.tile([1, 128], f32)
            nc.gpsimd.memset(ones_row[:], 1.0)
            ones_row_h = pers.tile([1, 128], bf16)
            nc.gpsimd.memset(ones_row_h[:], 1.0)
            blka = pers.tile([128, CT, H], bf16)
            nc.sync.dma_start(blka[:], VH["blk_a"])
            blkaf = pers.tile([128, CT, H], f32)
            nc.sync.dma_start(blkaf[:], VF["blk_af"])
            blkb = pers.tile([12, CT, 128], f32)
            nc.sync.dma_start(blkb[:], VF["blk_b"])
            eps_ln = pers.tile([128, 1], f32)
            nc.gpsimd.memset(eps_ln[:], LN_EPS)
            eps_gn = pers.tile([128, 1], f32)
            nc.gpsimd.memset(eps_gn[:], GN_EPS)
            emat_t = pers.tile([128, 2, Q], bf16)
            nc.sync.dma_start(emat_t[:], VH["emat"])
            mtri_t = pers.tile([128, 2, Q], bf16)
            nc.sync.dma_start(mtri_t[:], VH["mtri"])
            meye_t = pers.tile([128, 2, Q], bf16)
            nc.sync.dma_start(meye_t[:], VH["meye"])
            irow_t = pers.tile([128, Q], bf16)
            nc.sync.dma_start(irow_t[:], VH["irow"])

            def layernorm(src):
                xln = actp.tile([128, CT, T], bf16, tag="xln", bufs=1)
                for tch in range(2):
                    tsl = slice(tch * 512, (tch + 1) * 512)
                    mu_ps = pss.tile([128, 512], f32, tag="pss")
                    m2_ps = pss.tile([128, 512], f32, tag="pss")
                    for ct in range(CT):
                        nc.tensor.matmul(mu_ps[0:1, :], ones_col[:], src[:, ct, tsl],
                                         start=(ct == 0), stop=(ct == CT - 1))
                    for ct in range(CT):
                        sq = tmpp.tile([128, 512], f32, tag="lnsq", bufs=1)
                        nc.vector.tensor_mul(sq[:], src[:, ct, tsl], src[:, ct, tsl])
                        nc.tensor.matmul(m2_ps[0:1, :], ones_col[:], sq[:],
                                         start=(ct == 0), stop=(ct == CT - 1))
                    mu_row = sm.tile([1, 512], f32, tag="rows", bufs=3)
                    nc.scalar.activation(mu_row[:], mu_ps[0:1, :], AF.Copy,
                                         scale=1.0 / C)
                    mu2_row = sm.tile([1, 512], f32, tag="rows", bufs=3)
                    nc.vector.tensor_mul(mu2_row[:], mu_row[:], mu_row[:])
                    var_row = sm.tile([1, 512], f32, tag="rows", bufs=3)
                    nc.vector.scalar_tensor_tensor(
                        var_row[:], m2_ps[0:1, :], 1.0 / C, mu2_row[:],
                        OP.mult, OP.subtract)
                    std_row = sm.tile([1, 512], f32, tag="rows", bufs=3)
                    nc.scalar.activation(std_row[:], var_row[:], AF.Sqrt,
                                         bias=eps_ln[0:1, :])
                    rstd_row = sm.tile([1, 512], f32, tag="rows", bufs=3)
                    nc.vector.reciprocal(rstd_row[:], std_row[:])
                    MU = ps512.tile([128, 512], f32, tag="ps512")
                    RSTD = ps512.tile([128, 512], f32, tag="ps512")
                    nc.tensor.matmul(MU[:], ones_row[:], mu_row[:],
                                     start=True, stop=True)
                    nc.tensor.matmul(RSTD[:], ones_row[:], rstd_row[:],
                                     start=True, stop=True)
                    for ct in range(CT):
                        t = tmpp.tile([128, 512], f32, tag="lnsq", bufs=1)
                        nc.vector.tensor_sub(t[:], src[:, ct, tsl], MU[:])
                        nc.vector.tensor_mul(xln[:, ct, tsl], t[:], RSTD[:])
                return xln

            def mix(xln, maa_t, kind, tch):
                """m = xln*(1-maa) + shift(xln)*maa for tokens [tch*512, +512)"""
                m = mixp.tile([128, CT, 512], bf16, tag="mix")
                lo = tch * 512
                for ct in range(CT):
                    nc.vector.tensor_scalar_mul(
                        m[:, ct, :], xln[:, ct, lo:lo + 512],
                        maa_t[:, ct, 6 + kind:7 + kind])
                    if tch == 0:
                        nc.vector.scalar_tensor_tensor(
                            m[:, ct, 1:512], xln[:, ct, 0:511],
                            maa_t[:, ct, kind:kind + 1], m[:, ct, 1:512],
                            OP.mult, OP.add)
                    else:
                        nc.vector.scalar_tensor_tensor(
                            m[:, ct, :], xln[:, ct, lo - 1:lo + 511],
                            maa_t[:, ct, kind:kind + 1], m[:, ct, :],
                            OP.mult, OP.add)
                return m

            def load_w(name, l, fsl=None):
                t = wp.tile([128, CT, C], bf16, tag="wcc")
                if fsl is None:
                    nc.sync.dma_start(t[:], G[name][l])
                else:
                    nc.sync.dma_start(t[:], G[name][l, :, :, fsl])
                return t

            # ============== layers ==============
            for l in range(n_layers):
                maa_t = lcon.tile([128, CT, 12], f32, tag="maa")
                nc.sync.dma_start(maa_t[:], VF["maa"][l])
                bias_t = lcon.tile([128, CT, 8], f32, tag="bias")
                nc.sync.dma_start(bias_t[:], VF["bias_cm"][l])
                bckt = lcon.tile([128, HT], f32, tag="bck")
                nc.sync.dma_start(bckt[:], VF["bck_t"][l])
                lnwh_t = lcon.tile([128, H], f32, tag="lnwh")
                nc.sync.dma_start(lnwh_t[:], VF["lnw_h"][l])
                uh_t = lcon.tile([128, H], f32, tag="uh")
                nc.sync.dma_start(uh_t[:], VF["u_h"][l])
                lnwct_t = lcon.tile([128, CT], f32, tag="lnwct")
                nc.sync.dma_start(lnwct_t[:], VF["lnw_ct"][l])
                # decay tables: wmT[p,h,jt,i] = exp(min(lnw_h*(i-j-1), 0))
                #   * [i>j] + u_h*[i==j]  (j = jt*128+p);  wbq = exp(lnw*i)
                wmT = lcon.tile([128, H, 2, Q], bf16, tag="wmT")
                for h in range(H):
                    for jt in range(2):
                        es = tmpp.tile([128, Q], f32, tag="wmes", bufs=1)
                        nc.vector.tensor_scalar(
                            es[:], emat_t[:, jt, :], lnwh_t[:, h:h + 1], 0.0,
                            OP.mult, OP.min)
                        pe = tmpp.tile([128, Q], bf16, tag="wmpe", bufs=1)
                        nc.scalar.activation(pe[:], es[:], AF.Exp)
                        pm = tmpp.tile([128, Q], bf16, tag="wmpm", bufs=1)
                        nc.vector.tensor_mul(pm[:], pe[:], mtri_t[:, jt, :])
                        nc.vector.scalar_tensor_tensor(
                            wmT[:, h, jt, :], meye_t[:, jt, :],
                            uh_t[:, h:h + 1], pm[:], OP.mult, OP.add)
                wbq = lcon.tile([128, CT, Q], bf16, tag="wbq")
                for ct in range(CT):
                    es = tmpp.tile([128, Q], f32, tag="wmes", bufs=1)
                    nc.vector.tensor_scalar(
                        es[:], irow_t[:], lnwct_t[:, ct:ct + 1], 0.0,
                        OP.mult, OP.min)
                    nc.scalar.activation(wbq[:, ct, :], es[:], AF.Exp)
                wkc = lcon.tile([128, H * 2], f32, tag="wk")
                nc.sync.dma_start(wkc[:], VF["wk_col"][l])
                wsbc = lcon.tile([128, H], f32, tag="ws")
                nc.sync.dma_start(wsbc[:], VF["ws_bc"][l])
                brow0 = lcon.tile([1, C], bf16, tag="brow0")
                nc.sync.dma_start(brow0[:], VH["bias_rows"][l, 0:1, :])
                brow1 = lcon.tile([1, C], bf16, tag="brow1")
                nc.sync.dma_start(brow1[:], VH["bias_rows"][l, 1:2, :])

                # ---- tmix ----
                xln = layernorm(x_res)
                kC = actp.tile([128, CT, T], bf16, tag="kh", bufs=1)
                kT = actp.tile([128, TT, C], bf16, tag="kT", bufs=1)
                vT = actp.tile([128, TT, C], bf16, tag="vT", bufs=1)
                r_sb = actp.tile([128, CT, T], bf16, tag="r", bufs=1)
                gC = actp.tile([128, CT, T], bf16, tag="gx", bufs=1)

                def proj_cm_dst(mx, wt, dst, func, bkind):
                    for mt in range(CT):
                        for tch in range(2):
                            tsl = slice(tch * 512, (tch + 1) * 512)
                            ps = ps512.tile([128, 512], f32, tag="ps512")
                            for kt in range(CT):
                                nc.tensor.matmul(
                                    ps[:], wt[:, kt, mt * 128:(mt + 1) * 128],
                                    mx[tch][:, kt, :],
                                    start=(kt == 0), stop=(kt == CT - 1))
                            if func == "silu":
                                sg = tmpp.tile([128, 512], bf16, tag="sg", bufs=1)
                                nc.scalar.activation(sg[:], ps[:], AF.Sigmoid,
                                                     bias=bias_t[:, mt, bkind:bkind + 1])
                                nc.vector.scalar_tensor_tensor(
                                    dst[:, mt, tsl], ps[:],
                                    bias_t[:, mt, bkind:bkind + 1], sg[:],
                                    OP.add, OP.mult)
                            else:
                                nc.scalar.activation(dst[:, mt, tsl], ps[:], func,
                                                     bias=bias_t[:, mt, bkind:bkind + 1])

                def proj_tm_dst(mx, wt, dst, brow):
                    for tch in range(2):
                        for tt4 in range(4):
                            tt = tch * 4 + tt4
                            for nch in range(2):
                                nsl = slice(nch * 384, (nch + 1) * 384)
                                ps = ps512.tile([128, 512], f32, tag="ps512")
                                for kt in range(CT):
                                    nc.tensor.matmul(
                                        ps[:, 0:384],
                                        mx[tch][:, kt, tt4 * 128:(tt4 + 1) * 128],
                                        wt[:, kt, nsl], start=(kt == 0), stop=False)
                                nc.tensor.matmul(ps[:, 0:384], ones_row_h[:],
                                                 brow[0:1, nsl], start=False,
                                                 stop=True)
                                nc.scalar.activation(dst[:, tt, nsl], ps[:, 0:384],
                                                     AF.Copy)

                wk_t = load_w("Wk", l)
                xk = [mix(xln, maa_t, 0, 0), mix(xln, maa_t, 0, 1)]
                proj_cm_dst(xk, wk_t, kC, AF.Identity, 1)
                proj_tm_dst(xk, wk_t, kT, brow0)
                wv_t = load_w("Wv", l)
                xv = [mix(xln, maa_t, 1, 0), mix(xln, maa_t, 1, 1)]
                proj_tm_dst(xv, wv_t, vT, brow1)
                wr_t = load_w("Wr", l)
                xr = [mix(xln, maa_t, 2, 0), mix(xln, maa_t, 2, 1)]
                proj_cm_dst(xr, wr_t, r_sb, AF.Identity, 0)
                wg_t = load_w("Wg", l)
                xg = [mix(xln, maa_t, 3, 0), mix(xln, maa_t, 3, 1)]
                proj_cm_dst(xg, wg_t, gC, "silu", 6)

                # ---- chunked attention ----
                wo_t = load_w("Wo", l)
                for ch in range(NCH):
                    csl = slice(ch * Q, (ch + 1) * Q)
                    yT = chp.tile([128, CT, Q], bf16, tag="yT")
                    for ct in range(CT):
                        if ch > 0:
                            rwf = tmpp.tile([128, Q], bf16, tag="rwf", bufs=2)
                            nc.vector.tensor_mul(rwf[:], r_sb[:, ct, csl],
                                                 wbq[:, ct, :])
                        y_ps = ps512.tile([128, 512], f32, tag="ps512")
                        for hh in range(2):
                            h = 2 * ct + hh
                            hp = hh * 64
                            a_ps = ps512.tile([128, 512], f32, tag="ps512")
                            for jt in range(2):
                                nc.tensor.matmul(
                                    a_ps[:, jt * Q:(jt + 1) * Q],
                                    kC[hp:hp + 64, ct,
                                       (2 * ch + jt) * 128:(2 * ch + jt + 1) * 128],
                                    r_sb[hp:hp + 64, ct, csl],
                                    start=True, stop=True)
                            a_sb = tmpp.tile([128, 512], bf16, tag="attT", bufs=2)
                            for jt in range(2):
                                nc.vector.tensor_mul(
                                    a_sb[:, jt * Q:(jt + 1) * Q],
                                    a_ps[:, jt * Q:(jt + 1) * Q], wmT[:, h, jt, :])
                            ysl = y_ps[hp:hp + 64, 0:Q]
                            for jt in range(2):
                                nc.tensor.matmul(
                                    ysl, vT[:, 2 * ch + jt, h * 64:(h + 1) * 64],
                                    a_sb[:, jt * Q:(jt + 1) * Q],
                                    start=(jt == 0),
                                    stop=(jt == 1 and ch == 0))
                            if ch > 0:
                                nc.tensor.matmul(ysl, S_b[hp:hp + 64, h, :],
                                                 rwf[hp:hp + 64, :],
                                                 start=False, stop=True)
                            kw = tmpp.tile([128, 2, HD], bf16, tag="kw", bufs=1)
                            for jt in range(2):
                                nc.vector.tensor_scalar_mul(
                                    kw[:, jt, :],
                                    kT[:, 2 * ch + jt, h * 64:(h + 1) * 64],
                                    wkc[:, h * 2 + jt:h * 2 + jt + 1])
                            c_ps = pss.tile([128, 512], f32, tag="pss")
                            for jt in range(2):
                                nc.tensor.matmul(
                                    c_ps[hp:hp + 64, 0:64], kw[:, jt, :],
                                    vT[:, 2 * ch + jt, h * 64:(h + 1) * 64],
                                    start=(jt == 0), stop=(jt == 1))
                            if ch == 0:
                                nc.vector.tensor_copy(S_f[hp:hp + 64, h, :],
                                                      c_ps[hp:hp + 64, 0:64])
                            else:
                                nc.vector.scalar_tensor_tensor(
                                    S_f[hp:hp + 64, h, :], S_f[hp:hp + 64, h, :],
                                    wsbc[hp:hp + 64, h:h + 1],
                                    c_ps[hp:hp + 64, 0:64], OP.mult, OP.add)
                            if ch < NCH - 1:
                                nc.vector.tensor_copy(S_b[hp:hp + 64, h, :],
                                                      S_f[hp:hp + 64, h, :])
                        nc.scalar.activation(yT[:, ct, :], y_ps[:, 0:Q], AF.Copy)

                    # GroupNorm: two-pass (center in place, then sum of squares)
                    mu_ps = pss.tile([128, 512], f32, tag="pss")
                    for ct in range(CT):
                        nc.tensor.matmul(mu_ps[0:H, 0:Q], blka[:, ct, :], yT[:, ct, :],
                                         start=(ct == 0), stop=(ct == CT - 1))
                    mu_sb = sm.tile([12, Q], f32, tag="gvar", bufs=3)
                    nc.scalar.activation(mu_sb[:], mu_ps[0:H, 0:Q], AF.Copy,
                                         scale=1.0 / HD)
                    for ct in range(CT):
                        MUb = ps512.tile([128, 512], f32, tag="ps512")
                        nc.tensor.matmul(MUb[:, 0:Q], blkb[:, ct, :], mu_sb[:],
                                         start=True, stop=True)
                        nc.vector.tensor_sub(yT[:, ct, :], yT[:, ct, :], MUb[:, 0:Q])
                    m2_ps = pss.tile([128, 512], f32, tag="pss")
                    for ct in range(CT):
                        sq = tmpp.tile([128, Q], f32, tag="gnsq", bufs=2)
                        nc.vector.tensor_mul(sq[:], yT[:, ct, :], yT[:, ct, :])
                        nc.tensor.matmul(m2_ps[0:H, 0:Q], blkaf[:, ct, :], sq[:],
                                         start=(ct == 0), stop=(ct == CT - 1))
                    var_sb = sm.tile([12, Q], f32, tag="gvar", bufs=3)
                    nc.scalar.activation(var_sb[:], m2_ps[0:H, 0:Q], AF.Copy,
                                         scale=1.0 / HD)
                    std_sb = sm.tile([12, Q], f32, tag="gvar", bufs=3)
                    nc.scalar.activation(std_sb[:], var_sb[:], AF.Sqrt,
                                         bias=eps_gn[0:12, :])
                    rstd_sb = sm.tile([12, Q], f32, tag="gvar", bufs=3)
                    nc.vector.reciprocal(rstd_sb[:], std_sb[:])
                    prod = chp.tile([128, CT, Q], bf16, tag="prod")
                    for ct in range(CT):
                        RSb = ps512.tile([128, 512], f32, tag="ps512")
                        nc.tensor.matmul(RSb[:, 0:Q], blkb[:, ct, :], rstd_sb[:],
                                         start=True, stop=True)
                        t2 = tmpp.tile([128, Q], f32, tag="gnt", bufs=2)
                        nc.vector.tensor_mul(t2[:], yT[:, ct, :], RSb[:, 0:Q])
                        nc.vector.scalar_tensor_tensor(
                            prod[:, ct, :], t2[:], bias_t[:, ct, 5:6],
                            gC[:, ct, csl], OP.add, OP.mult)
                    for mt in range(CT):
                        ps = ps512.tile([128, 512], f32, tag="ps512")
                        for kt in range(CT):
                            nc.tensor.matmul(
                                ps[:, 0:Q], wo_t[:, kt, mt * 128:(mt + 1) * 128],
                                prod[:, kt, :], start=(kt == 0), stop=(kt == CT - 1))
                        nc.vector.scalar_tensor_tensor(
                            x_res[:, mt, csl], ps[:, 0:Q], bias_t[:, mt, 2:3],
                            x_res[:, mt, csl], OP.add, OP.add)

                # ---- cmix ----
                xln2 = layernorm(x_res)
                wcr_t = load_w("Wcr", l)
                xr2 = [mix(xln2, maa_t, 5, 0), mix(xln2, maa_t, 5, 1)]
                gate = actp.tile([128, CT, T], bf16, tag="gx", bufs=1)
                for mt in range(CT):
                    for tch in range(2):
                        tsl = slice(tch * 512, (tch + 1) * 512)
                        ps = ps512.tile([128, 512], f32, tag="ps512")
                        for kt in range(CT):
                            nc.tensor.matmul(
                                ps[:], wcr_t[:, kt, mt * 128:(mt + 1) * 128],
                                xr2[tch][:, kt, :],
                                start=(kt == 0), stop=(kt == CT - 1))
                        nc.scalar.activation(gate[:, mt, tsl], ps[:], AF.Sigmoid,
                                             bias=bias_t[:, mt, 4:5])
                xk2 = [mix(xln2, maa_t, 4, 0), mix(xln2, maa_t, 4, 1)]
                for tch in range(2):
                    tsl = slice(tch * 512, (tch + 1) * 512)
                    h2 = actp.tile([128, HT, 512], bf16, tag="kh", bufs=1)
                    for third in range(3):
                        wck_t = load_w("Wck", l, slice(third * C, (third + 1) * C))
                        for mt6 in range(6):
                            gmt = third * 6 + mt6
                            ps = ps512.tile([128, 512], f32, tag="ps512")
                            for kt in range(CT):
                                nc.tensor.matmul(
                                    ps[:], wck_t[:, kt, mt6 * 128:(mt6 + 1) * 128],
                                    xk2[tch][:, kt, :],
                                    start=(kt == 0), stop=(kt == CT - 1))
                            hr = tmpp.tile([128, 512], bf16, tag="hrelu", bufs=2)
                            nc.vector.tensor_scalar(
                                hr[:], ps[:], bckt[:, gmt:gmt + 1], 0.0,
                                OP.add, OP.max)
                            nc.vector.tensor_mul(h2[:, gmt, :], hr[:], hr[:])
                    for third in range(3):
                        wcv_t = wp.tile([128, HT, Q], bf16, tag="wcc")
                        nc.sync.dma_start(
                            wcv_t[:],
                            G["Wcv"][l, :, :, third * Q:(third + 1) * Q])
                        for mt2 in range(2):
                            gmt = third * 2 + mt2
                            ps = ps512.tile([128, 512], f32, tag="ps512")
                            for kt in range(HT):
                                nc.tensor.matmul(
                                    ps[:], wcv_t[:, kt, mt2 * 128:(mt2 + 1) * 128],
                                    h2[:, kt, :], start=(kt == 0), stop=(kt == HT - 1))
                            t = tmpp.tile([128, 512], f32, tag="cvt", bufs=1)
                            nc.vector.scalar_tensor_tensor(
                                t[:], ps[:], bias_t[:, gmt, 3:4], gate[:, gmt, tsl],
                                OP.add, OP.mult)
                            nc.vector.tensor_add(x_res[:, gmt, tsl],
                                                 x_res[:, gmt, tsl], t[:])

            # ============== head ==============
            sq_sb = sm.tile([128, CT, 2], f32, tag="hsq")
            for ct in range(CT):
                nc.vector.tensor_mul(sq_sb[:, ct, 1:2], x_res[:, ct, T - 1:T],
                                     x_res[:, ct, T - 1:T])
                nc.vector.tensor_copy(sq_sb[:, ct, 0:1], x_res[:, ct, T - 1:T])
            mu_ps = pss.tile([128, 512], f32, tag="pss")
            for ct in range(CT):
                nc.tensor.matmul(mu_ps[0:1, 0:2], ones_col[:], sq_sb[:, ct, :],
                                 start=(ct == 0), stop=(ct == CT - 1))
            st_row = sm.tile([1, 2], f32, tag="hrow", bufs=4)
            nc.scalar.activation(st_row[:], mu_ps[0:1, 0:2], AF.Copy, scale=1.0 / C)
            mu2_row = sm.tile([1, 1], f32, tag="hrow", bufs=4)
            nc.vector.tensor_mul(mu2_row[:], st_row[:, 0:1], st_row[:, 0:1])
            var_row = sm.tile([1, 1], f32, tag="hrow", bufs=4)
            nc.vector.tensor_sub(var_row[:], st_row[:, 1:2], mu2_row[:])
            stdh_row = sm.tile([1, 1], f32, tag="hrow", bufs=4)
            nc.scalar.activation(stdh_row[:], var_row[:], AF.Sqrt,
                                 bias=eps_ln[0:1, :])
            rstd_row = sm.tile([1, 1], f32, tag="hrow", bufs=4)
            nc.vector.reciprocal(rstd_row[:], stdh_row[:])
            MU128 = sm.tile([128, 1], f32, tag="hb")
            RSTD128 = sm.tile([128, 1], f32, tag="hb")
            nc.gpsimd.partition_broadcast(MU128[:], st_row[:, 0:1])
            nc.gpsimd.partition_broadcast(RSTD128[:], rstd_row[:])
            xl = sm.tile([128, CT], bf16, tag="xl")
            for ct in range(CT):
                nc.vector.scalar_tensor_tensor(
                    xl[:, ct:ct + 1], x_res[:, ct, T - 1:T], MU128[:],
                    RSTD128[:], OP.subtract, OP.mult)
            nv = (V + 511) // 512
            for nt in range(nv):
                nsz = min(512, V - nt * 512)
                ps = pss.tile([128, 512], f32, tag="pss")
                for kt in range(CT):
                    wv_sb = tmpp.tile([128, 512], bf16, tag="hw", bufs=2)
                    nc.sync.dma_start(
                        wv_sb[:, 0:nsz],
                        G["wteT"][:, kt, nt * 512:nt * 512 + nsz])
                    nc.tensor.matmul(ps[0:1, 0:nsz], xl[:, kt:kt + 1],
                                     wv_sb[:, 0:nsz],
                                     start=(kt == 0), stop=(kt == CT - 1))
                ot = sm.tile([1, 512], f32, tag="hout")
                nc.scalar.activation(ot[:, 0:nsz], ps[0:1, 0:nsz], AF.Copy)
                nc.sync.dma_start(out_logits.ap()[:, nt * 512:nt * 512 + nsz],
                                  ot[:, 0:nsz])

    nc.compile()
    return nc


def _get_program(n_layers=L):
    if n_layers not in _PROG_CACHE:
        _PROG_CACHE[n_layers] = _build_program(n_layers)
    return _PROG_CACHE[n_layers]


def kernel(**inputs):
    from concourse.bass_utils import run_bass_kernel_spmd

    in_maps, lbias = _host_precompute(inputs)
    nc = _get_program(L)
    res = run_bass_kernel_spmd(nc, in_maps, core_ids=list(range(NCORES)),
                               trace=False)
    out = np.zeros((B, 1, V), np.float32)
    for b in range(B):
        out[b, 0, :] = res.results[b]["logits"][0]
    out += lbias[None, None, :]
    return out
